# revision 1
# baseline (speedup 1.0000x reference)
"""Trainium2 Bass kernel for a MAGNA-KG message-passing layer.

Problem: N=50000 nodes, E=800000 edges, R=500 relations, D=256 dims,
H=8 heads, 3 PPR hops.  SPMD across 8 NeuronCores.

Sharding (edge parallelism):
  * nodes range-sharded: core c owns rows [c*NB, (c+1)*NB), NB=6272
  * edges sharded by owner of dst; within a core edges are grouped per
    PAIR of 128-node dst blocks as [b0-even b1-even | b0-odd b1-odd]
    (even/odd by table-row parity -> int16 gather indices), padded to
    multiples of 128; per-(block,parity) chunk counts maxed over cores
    at build time so the SPMD instruction stream is identical.
  * per hop, per pair: one even + one odd dma_gather of src rows from a
    replicated bf16 table, 0/1 indicator one-hots built ON DEVICE with a
    DVE iota/is_equal compare, per-edge attention scaling as a single
    broadcast multiply (head-major columns -> stride-1 runs of 32),
    segment-sum via indicator matmuls in PSUM, blend with resident
    alpha*feat0, table republished with a 2-piece AllGather that
    overlaps the block loop.
  * hop0 additionally gathers eh (packed in the 768B table rows) and er
    (256B rows); et comes from an indicator-transpose matmul; the
    softmax denominator rides the aggregation matmul for free (ex is
    written over the dead eh columns so rhs cols 256:264 sum to den).
  * the feed-forward tail is fused into hop2's pair loop; activation
    functions are phase-batched so the ACT table set never thrashes.

kernel(**inputs) takes FULL inputs and returns the FULL [N, 256] output.
"""

import numpy as np
import ml_dtypes

import concourse.bacc as bacc
import concourse.bass as bass
import concourse.mybir as mybir
from concourse import tile

F32 = mybir.dt.float32
BF16 = mybir.dt.bfloat16
FP8 = mybir.dt.float8e4
I16 = mybir.dt.int16
ALU = mybir.AluOpType
ACTF = mybir.ActivationFunctionType

BF = ml_dtypes.bfloat16

NEG_BIG = -1.0e9


class Cfg:
    def __init__(self, N=50000, E=800000, R=500, P=8, HOPS=3,
                 ALPHA=0.15, SLOPE=0.2, EPS=1e-5):
        self.N, self.E, self.R, self.P = N, E, R, P
        self.D, self.H, self.AD = 256, 8, 32
        self.HOPS, self.ALPHA, self.SLOPE, self.EPS = HOPS, ALPHA, SLOPE, EPS
        self.B = -(-N // (P * 128))          # 49 dst blocks of 128 per core
        self.NB = self.B * 128               # 6272 nodes per core (padded)
        self.NPAD = P * self.NB              # 50176
        self.RPAD = 512
        self.R_SENT = R                      # sentinel er row (NEG_BIG)
        # AllGather pieces (block-aligned row ranges of the local slab)
        self.PIECES = ((0, 2560), (2560, 4864), (4864, self.B * 128))
        self.SC0 = 384                       # hop0 row: [feat0(256)|eh(8)|pad]


def _cdiv(a, b):
    return -(-a // b)


# --------------------------------------------------------------------------
# host-side planning
# --------------------------------------------------------------------------

class Plan:
    pass


def row_of(cfg, node):
    """Global row in the piece-split, rank-major AllGather table layout."""
    r = node // cfg.NB
    l = node % cfg.NB
    out = np.zeros_like(node)
    base = 0
    for (p0, p1) in cfg.PIECES:
        w = p1 - p0
        m = (l >= p0) & (l < p1)
        out = np.where(m, base + r * w + (l - p0), out)
        base += cfg.P * w
    return out


def plan_edges(cfg, src, dst, eids):
    P, B, NB = cfg.P, cfg.B, cfg.NB
    src = np.asarray(src).astype(np.int64)
    dst = np.asarray(dst).astype(np.int64)
    eids = np.asarray(eids).astype(np.int64)

    core_of = dst // NB
    per_core = []
    cnts = np.zeros((P, B, 2), np.int64)
    for c in range(P):
        m = core_of == c
        s, d, r = src[m], dst[m], eids[m]
        blk = (d - c * NB) // 128
        row = row_of(cfg, s)
        par = row & 1
        order = np.lexsort((par, blk))
        s, d, r, row, blk, par = (s[order], d[order], r[order], row[order],
                                  blk[order], par[order])
        grp = blk * 2 + par
        cnt = np.bincount(grp, minlength=B * 2).reshape(B, 2)
        cnts[c] = cnt
        per_core.append((s, d, r, row, grp, cnt))

    K = np.maximum(_cdiv(cnts, 128).max(axis=0), 1)      # [B, 2] chunks

    # pairs of blocks; slot layout per pair: [evens of blocks | odds]
    pairs = [tuple(range(b, min(b + 2, B))) for b in range(0, B, 2)]
    pair_info = []
    gstart = np.zeros((B, 2), np.int64)   # slot start of (block, parity)
    ch = 0
    for bs in pairs:
        c0 = ch
        ke = int(sum(K[b, 0] for b in bs))
        ko = int(sum(K[b, 1] for b in bs))
        off = c0
        blk_chunks = {b: [] for b in bs}
        for b in bs:                       # even groups
            gstart[b, 0] = off * 128
            blk_chunks[b] += list(range(off, off + int(K[b, 0])))
            off += int(K[b, 0])
        for b in bs:                       # odd groups
            gstart[b, 1] = off * 128
            blk_chunks[b] += list(range(off, off + int(K[b, 1])))
            off += int(K[b, 1])
        ch = off
        pair_info.append(dict(bs=bs, c0=c0, ke=ke, ko=ko, kb=ke + ko,
                              blk_chunks=blk_chunks))
    TOTCH = ch
    TOT = TOTCH * 128

    pl = Plan()
    pl.K, pl.TOTCH, pl.TOT, pl.pairs = K, TOTCH, TOT, pair_info
    pl.cores = []
    for c in range(P):
        s, d, r, row, grp, cnt = per_core[c]
        starts = np.concatenate([[0], np.cumsum(cnt.reshape(-1))])[:-1]
        rank = np.arange(len(s)) - np.repeat(starts, cnt.reshape(-1))
        q = gstart.reshape(-1)[grp] + rank               # slot per edge

        feat_idx = np.zeros(TOT, np.int16)
        er_idx = np.full(TOT, cfg.R_SENT >> 1, np.int16)
        feat_idx[q] = (row >> 1).astype(np.int16)
        er_idx[q] = (r >> 1).astype(np.int16)
        m1 = np.zeros(TOT, np.float32)
        m1[q] = (r & 1).astype(np.float32)

        FP8NP = ml_dtypes.float8_e4m3
        ind = np.zeros((128, TOT), FP8NP)
        ind_T = np.zeros((128, TOT), FP8NP)
        lane = q % 128
        qch = q // 128
        dr = (d - c * NB) % 128
        ind[lane, qch * 128 + dr] = FP8NP(1.0)
        ind_T[dr, qch * 128 + lane] = FP8NP(1.0)

        def wrap(a):
            w = a.reshape(-1, 16).T                      # [16, TOT/16]
            return np.tile(w, (8, 1)).copy()             # [128, TOT/16]

        core = Plan()
        core.feat_idx = wrap(feat_idx)
        core.er_idx = wrap(er_idx)
        core.ind = ind
        core.ind_T = ind_T
        core.m1 = m1.reshape(TOTCH, 128).T.astype(BF).copy()   # [128, TOTCH]
        core.m0 = (1.0 - m1).reshape(TOTCH, 128).T.astype(BF).copy()
        pl.cores.append(core)
    return pl


# --------------------------------------------------------------------------
# bass program
# --------------------------------------------------------------------------

def build_nc(cfg, pl):
    P, B, NB, NPAD = cfg.P, cfg.B, cfg.NB, cfg.NPAD
    D, H, AD = cfg.D, cfg.H, cfg.AD
    SC0 = cfg.SC0
    TOTCH, TOT = pl.TOTCH, pl.TOT
    TOT16 = TOT // 16
    RG = [list(range(P))]

    nc = bacc.Bacc(None, target_bir_lowering=False, debug=False,
                   num_swdge_queues=4)
    shared = "Shared"

    def inp(name, shape, dtype):
        return nc.dram_tensor(name, shape, dtype, kind="ExternalInput")

    # ---- inputs -----------------------------------------------------------
    ent_own = inp("ent_own", [NB, D], F32)
    rel_pad = inp("rel_pad", [cfg.RPAD, D], F32)
    idx_feat = inp("idx_feat", [128, TOT16], I16)
    idx_er = inp("idx_er", [128, TOT16], I16)
    ind_in = inp("ind_in", [128, TOT], FP8)
    indT_in = inp("indT_in", [128, TOT], FP8)
    m0_in = inp("m0_in", [128, TOTCH], BF16)
    m1_in = inp("m1_in", [128, TOTCH], BF16)
    w_head = inp("w_head", [D, D], BF16)
    w_tail = inp("w_tail", [D, D], BF16)
    w_ent = inp("w_ent", [D, D], BF16)
    w_rel = inp("w_rel", [D, D], BF16)
    a_h = inp("a_h", [D, H], BF16)
    a_t = inp("a_t", [D, H], BF16)
    a_r = inp("a_r", [D, H], BF16)
    w_out = inp("w_out", [D, D], BF16)
    w1 = inp("w1", [D, 4 * D], BF16)
    w2 = inp("w2", [4 * D, D], BF16)
    g_e = inp("g_e", [128, D], F32)
    be_e = inp("be_e", [128, D], F32)
    g_r = inp("g_r", [128, D], F32)
    be_r = inp("be_r", [128, D], F32)
    g_ff = inp("g_ff", [128, D], F32)
    be_ff = inp("be_ff", [128, D], F32)
    b1t = inp("b1t", [128, 8], F32)
    b2r = inp("b2r", [128, D], F32)
    ident_in = inp("ident_in", [128, 128], BF16)

    out_rows = nc.dram_tensor("out_rows", [NB, D], F32, kind="ExternalOutput")

    # ---- internal DRAM ----------------------------------------------------
    er_tbl = nc.dram_tensor("er_tbl", [cfg.RPAD // 2, 256], BF16)
    feat0s_d = nc.dram_tensor("feat0s_d", [NB, D], BF16)
    slab0 = nc.dram_tensor("slab0", [NB, SC0], BF16)
    slab1 = nc.dram_tensor("slab1", [NB, D], BF16)
    slab2 = nc.dram_tensor("slab2", [NB, D], BF16)
    tbl0 = nc.dram_tensor("tbl0", [NPAD, SC0], BF16, addr_space=shared)
    tbl1 = nc.dram_tensor("tbl1", [NPAD, D], BF16, addr_space=shared)
    tbl2 = nc.dram_tensor("tbl2", [NPAD, D], BF16, addr_space=shared)
    slabs = [slab0, slab1, slab2]
    tbls = [tbl0, tbl1, tbl2]

    with tile.TileContext(nc, num_cores=P) as tc:
        with (
            tc.tile_pool(name="consts", bufs=1) as cp,
            tc.tile_pool(name="work", bufs=2) as wp,
            tc.tile_pool(name="gath", bufs=2) as gp,
            tc.tile_pool(name="gath3", bufs=3) as gp3,
            tc.tile_pool(name="pagg", bufs=2, space="PSUM") as pagg,
            tc.tile_pool(name="pmid", bufs=2, space="PSUM") as pmid,
            tc.tile_pool(name="ptps", bufs=2, space="PSUM") as ptps,
        ):
            from concourse import library_config
            nc.gpsimd.load_library(library_config.mlp)

            # ---- resident constants --------------------------------------
            def load_const(name, dram, shape, dtype):
                t = cp.tile(shape, dtype, name=name)
                nc.sync.dma_start(t[:], dram[:, :])
                return t

            ident = load_const("identc", ident_in, [128, 128], BF16)
            m0c = load_const("m0c", m0_in, [128, TOTCH], BF16)
            m1c = load_const("m1c", m1_in, [128, TOTCH], BF16)

            def load_w(name, dram, cols):
                t = cp.tile([128, D // 128, cols], BF16, name=name)
                nc.sync.dma_start(
                    t[:], dram.ap().rearrange("(kt p) c -> p kt c", p=128))
                return t

            whc = load_w("whc", w_head, D)
            wtc = load_w("wtc", w_tail, D)
            wec = load_w("wec", w_ent, D)
            wrc = load_w("wrc", w_rel, D)
            ahc = load_w("ahc", a_h, H)
            atc = load_w("atc", a_t, H)
            arc = load_w("arc", a_r, H)
            woc = load_w("woc", w_out, D)
            w1c = load_w("w1c", w1, 4 * D)
            w2c = cp.tile([128, 4 * D // 128, D], BF16, name="w2c")
            nc.sync.dma_start(
                w2c[:], w2.ap().rearrange("(kt p) c -> p kt c", p=128))
            gec = load_const("gec", g_e, [128, D], F32)
            bec = load_const("bec", be_e, [128, D], F32)
            grc = load_const("grc", g_r, [128, D], F32)
            brc = load_const("brc", be_r, [128, D], F32)
            gfc = load_const("gfc", g_ff, [128, D], F32)
            bfc = load_const("bfc", be_ff, [128, D], F32)
            b1c = load_const("b1c", b1t, [128, 8], F32)
            b2c = load_const("b2c", b2r, [128, D], F32)

            ex_sb = cp.tile([128, TOTCH, 8], BF16, name="ex_sb")
            rden_sb = cp.tile([128, B, 8], F32, name="rden_sb")
            et_own = cp.tile([128, B, 8], BF16, name="et_own")
            hr_all = cp.tile([128, cfg.RPAD // 128, D], BF16, name="hr_all")
            eps_t = cp.tile([128, 1], F32, name="eps_t")
            nc.vector.memset(eps_t[:], cfg.EPS)
            lg_t = cp.tile([128, 1], F32, name="lg_t")
            nc.vector.memset(lg_t[:], float(-np.log(1.0 - cfg.ALPHA)))

            # ------------------------------------------------------------------
            # helpers
            # ------------------------------------------------------------------
            def ln(x_f32, gamma, beta, out_t):
                st = wp.tile([128, 6], F32, name="ln_st", tag="ln_st")
                ag = wp.tile([128, 2], F32, name="ln_ag", tag="ln_ag")
                sd = wp.tile([128, 1], F32, name="ln_sd", tag="ln_sd")
                rv = wp.tile([128, 1], F32, name="ln_rv", tag="ln_rv")
                xc = wp.tile([128, D], F32, name="ln_xc", tag="ln_xc")
                nc.vector.bn_stats(st[:], x_f32)
                nc.vector.bn_aggr(ag[:], st[:])
                nc.scalar.activation(sd[:], ag[:, 1:2], ACTF.Sqrt,
                                     bias=eps_t[:])
                nc.vector.reciprocal(rv[:], sd[:])
                nc.vector.tensor_scalar(xc[:], x_f32, ag[:, 0:1], rv[:],
                                        ALU.subtract, ALU.mult)
                nc.vector.scalar_tensor_tensor(
                    xc[:], xc[:], 1.0, gamma, ALU.mult, ALU.mult)
                nc.vector.tensor_tensor(out_t, xc[:], beta, ALU.add)

            def transpose_2(src_bf16, name):
                """[128, D] bf16 -> [128, 2, 128] transposed k-tiles."""
                t = wp.tile([128, D // 128, 128], BF16, name=name, tag="tps_o")
                for k in range(D // 128):
                    ps = ptps.tile([128, 128], BF16, name="tps_ps", tag="tps")
                    nc.tensor.transpose(
                        ps[:], src_bf16[:, k * 128:(k + 1) * 128], ident[:])
                    nc.vector.tensor_copy(t[:, k, :], ps[:])
                return t

            def gather(out_t, tbl_view, idx_dram, q0, n, elem, estep, name):
                it = gp.tile([128, n // 16], I16, name=name, tag=name)
                nc.sync.dma_start(it[:], idx_dram[:, q0 // 16:(q0 + n) // 16])
                nc.gpsimd.dma_gather(out_t, tbl_view, it[:], n, n, elem,
                                     elem_step=estep, single_packet=False)

            # ------------------------------------------------------------------
            # P0a/P1a: LayerNorms (ACT set: sqrt)
            # ------------------------------------------------------------------
            with nc.named_scope("p1a"):
                negt = wp.tile([128, 256], BF16, name="negt", tag="negt")
                nc.vector.memset(negt[:], NEG_BIG)
                for i in range(cfg.RPAD // 256):
                    nc.sync.dma_start(er_tbl[i * 128:(i + 1) * 128, :],
                                      negt[:])

                for i in range(cfg.RPAD // 128):
                    xr = wp.tile([128, D], F32, name="xr", tag="x_in")
                    nc.sync.dma_start(xr[:], rel_pad[i * 128:(i + 1) * 128, :])
                    ln(xr[:], grc[:], brc[:], hr_all[:, i, :])

                he_all = gp3.tile([128, B, D], BF16, name="he_all",
                                  tag="gb")
                for i in range(B):
                    xe = wp.tile([128, D], F32, name="xe", tag="x_in")
                    nc.sync.dma_start(xe[:], ent_own[i * 128:(i + 1) * 128, :])
                    ln(xe[:], gec[:], bec[:], he_all[:, i, :])

            # ------------------------------------------------------------------
            # P0b: relation path -> er_tbl  (ACT set: tanh/exp)
            # ------------------------------------------------------------------
            with nc.named_scope("p0b"):
                for i in range(cfg.RPAD // 128):
                    rows0 = i * 128
                    nrows = min(cfg.R - rows0, 128) if rows0 < cfg.R else 0
                    if nrows <= 0:
                        continue
                    hrt = transpose_2(hr_all[:, i, :], "hrt")
                    tht = wp.tile([128, D // 128, 128], BF16, name="tht",
                                  tag="tht")
                    for o in range(D // 128):
                        ps = ptps.tile([128, 128], F32, name="pp", tag="tps")
                        for k in range(D // 128):
                            nc.tensor.matmul(
                                ps[:], wrc[:, k, o * 128:(o + 1) * 128],
                                hrt[:, k, :], start=(k == 0),
                                stop=(k == D // 128 - 1))
                        nc.scalar.activation(tht[:, o, :], ps[:], ACTF.Tanh)
                    erp = pmid.tile([16, 128], F32, name="erp", tag="pmid")
                    for o in range(D // 128):
                        nc.tensor.matmul(erp[0:8, :], arc[:, o, :],
                                         tht[:, o, :], start=(o == 0),
                                         stop=(o == D // 128 - 1))
                    ers = wp.tile([16, 128], BF16, name="ers", tag="ers")
                    nc.vector.tensor_copy(ers[0:8, :], erp[0:8, :])
                    ept = ptps.tile([128, 128], BF16, name="ept", tag="tps")
                    nc.tensor.transpose(ept[:, 0:8], ers[0:8, :],
                                        ident[0:8, 0:8])
                    erv = wp.tile([128, 8], BF16, name="erv", tag="erv")
                    nc.vector.tensor_copy(erv[:], ept[:, 0:8])
                    er_v2 = er_tbl.ap().rearrange("n (two c) -> (n two) c",
                                                  two=2)
                    nc.sync.dma_start(
                        er_v2[rows0:rows0 + nrows, 0:8], erv[0:nrows, :])

            # ------------------------------------------------------------------
            # P1b: head — projections, eh/et, feat0  (ACT set: tanh)
            # ------------------------------------------------------------------
            with nc.named_scope("p1b"):
                for i in range(B):
                    het = transpose_2(he_all[:, i, :], "het")
                    f0r = wp.tile([128, SC0], BF16, name="f0r", tag="f0r")
                    nc.vector.memset(f0r[:, D:], 0.0)
                    for (wc, ac, sl) in ((whc, ahc, 0), (wtc, atc, 1)):
                        tht = wp.tile([128, D // 128, 128], BF16, name="thx",
                                      tag="tht")
                        for o in range(D // 128):
                            ps = ptps.tile([128, 128], F32, name="pp",
                                           tag="tps")
                            for k in range(D // 128):
                                nc.tensor.matmul(
                                    ps[:], wc[:, k, o * 128:(o + 1) * 128],
                                    het[:, k, :], start=(k == 0),
                                    stop=(k == D // 128 - 1))
                            nc.scalar.activation(tht[:, o, :], ps[:],
                                                 ACTF.Tanh)
                        ap_ps = pmid.tile([16, 128], F32, name="ap_ps",
                                          tag="pmid")
                        for o in range(D // 128):
                            nc.tensor.matmul(ap_ps[0:8, :], ac[:, o, :],
                                             tht[:, o, :], start=(o == 0),
                                             stop=(o == D // 128 - 1))
                        aps = wp.tile([8, 128], BF16, name="aps", tag="ers")
                        nc.vector.tensor_copy(aps[:], ap_ps[0:8, :])
                        spt = ptps.tile([128, 128], BF16, name="spt",
                                        tag="tps")
                        nc.tensor.transpose(spt[:, 0:8], aps[:],
                                            ident[0:8, 0:8])
                        if sl == 0:
                            nc.vector.tensor_copy(f0r[:, D:D + 8],
                                                  spt[:, 0:8])
                        else:
                            nc.vector.tensor_copy(et_own[:, i, :],
                                                  spt[:, 0:8])

                    f0t = wp.tile([128, D // 128, 128], BF16, name="f0t",
                                  tag="tht")
                    for o in range(D // 128):
                        ps = ptps.tile([128, 128], F32, name="fp", tag="tps")
                        for k in range(D // 128):
                            nc.tensor.matmul(
                                ps[:], wec[:, k, o * 128:(o + 1) * 128],
                                het[:, k, :], start=(k == 0),
                                stop=(k == D // 128 - 1))
                        nc.vector.tensor_copy(f0t[:, o, :], ps[:])
                    for o in range(D // 128):
                        ps = ptps.tile([128, 128], BF16, name="fr", tag="tps")
                        nc.tensor.transpose(ps[:], f0t[:, o, :], ident[:])
                        nc.vector.tensor_copy(f0r[:, o * 128:(o + 1) * 128],
                                              ps[:])
                    nc.sync.dma_start(slab0[i * 128:(i + 1) * 128, :], f0r[:])
                    f0s = wp.tile([128, D], BF16, name="f0s", tag="f0s")
                    nc.vector.tensor_scalar_mul(f0s[:], f0r[:, 0:D],
                                                cfg.ALPHA)
                    nc.sync.dma_start(feat0s_d[i * 128:(i + 1) * 128, :],
                                      f0s[:])
                    for pno, (p0, p1) in enumerate(cfg.PIECES):
                        if i == B - 1 and False:
                            w = p1 - p0
                            base = P * sum(q1 - q0 for (q0, q1)
                                           in cfg.PIECES[:pno])
                            nc.gpsimd.collective_compute(
                                "AllGather", ALU.bypass, replica_groups=RG,
                                ins=[slab0.ap()[p0:p1, :].opt()],
                                outs=[tbl0.ap()
                                      [base:base + P * w, :].opt()])
            for pno, (p0, p1) in enumerate(cfg.PIECES):
                w = p1 - p0
                base = P * sum(q1 - q0 for (q0, q1) in cfg.PIECES[:pno])
                nc.gpsimd.collective_compute(
                    "AllGather", ALU.bypass, replica_groups=RG,
                    ins=[slab0.ap()[p0:p1, :].opt()],
                    outs=[tbl0.ap()[base:base + P * w, :].opt()])
            er_v = er_tbl.ap()

            # ------------------------------------------------------------------
            # hops
            # ------------------------------------------------------------------
            npairs = len(pl.pairs)
            for t in range(cfg.HOPS):
                W = SC0 if t == 0 else D
                tb_v = tbls[t].ap().rearrange("(n two) c -> n (two c)", two=2)
                tb_even, tb_odd = tb_v[:, 0:W], tb_v[:, W:2 * W]
                last = t + 1 == cfg.HOPS
                with nc.named_scope(f"hop{t}"):
                    for pi, pr in enumerate(pl.pairs):
                        bs, c0, ke, ko, kb = (pr['bs'], pr['c0'], pr['ke'],
                                              pr['ko'], pr['kb'])
                        nb = len(bs)
                        q0 = c0 * 128

                        gb = gp3.tile([128, kb, W], BF16, name="gb",
                                      tag="gb")
                        gather(gb[:, 0:ke, :], tb_even, idx_feat, q0,
                               ke * 128, W, 2 * W, "ix_f0")
                        gather(gb[:, ke:kb, :], tb_odd, idx_feat,
                               q0 + ke * 128, ko * 128, W, 2 * W, "ix_f1")

                        ind_t = gp.tile([128, kb, 128], FP8, name="ind_t",
                                        tag="ind_t")
                        nc.sync.dma_start(
                            ind_t[:].rearrange("p k l -> p (k l)"),
                            ind_in[:, q0:q0 + kb * 128])

                        if t == 0:
                            kh0 = (kb + 1) // 2
                            for s0 in range(0, kb, kh0):
                                s1 = min(s0 + kh0, kb)
                                ks = s1 - s0
                                ca, cz = c0 + s0, c0 + s1
                                qa = ca * 128
                                indT_t = gp.tile([128, kh0 * 128], FP8,
                                                 name="indT_t", tag="indT_t")
                                nc.sync.dma_start(
                                    indT_t[:, 0:ks * 128],
                                    indT_in[:, qa:qa + ks * 128])
                                erg = gp.tile([128, kh0, 256], BF16,
                                              name="erg", tag="erg")
                                gather(erg[:, 0:ks, :], er_v, idx_er, qa,
                                       ks * 128, 256, 256, "ix_er")
                                et_ps = pmid.tile([128, kh0, 8], F32,
                                                  name="et_ps", tag="pmid")
                                for ci in range(ks):
                                    b_of = next(
                                        b for b in bs
                                        if ca + ci in pr['blk_chunks'][b])
                                    nc.tensor.matmul(
                                        et_ps[:, ci, :],
                                        indT_t[:, ci * 128:(ci + 1) * 128],
                                        et_own[:, b_of, :],
                                        start=True, stop=True)
                                sc_s = wp.tile([128, kh0, 8], F32,
                                               name="sc_s", tag="sc_s")
                                nc.vector.tensor_tensor(
                                    sc_s[:, 0:ks, :], gb[:, s0:s1, D:D + 8],
                                    et_ps[:, 0:ks, :], ALU.add)
                                # er = er_hi + (er_lo - er_hi) * m0
                                erd = wp.tile([128, kh0, 8], BF16,
                                              name="erd", tag="erd")
                                nc.vector.tensor_tensor(
                                    erd[:, 0:ks, :], erg[:, 0:ks, 0:8],
                                    erg[:, 0:ks, 128:136], ALU.subtract)
                                nc.vector.tensor_tensor(
                                    erd[:, 0:ks, :], erd[:, 0:ks, :],
                                    m0c[:, ca:cz].unsqueeze(2)
                                    .broadcast_to([128, ks, 8]), ALU.mult)
                                nc.vector.tensor_tensor(
                                    sc_s[:, 0:ks, :], sc_s[:, 0:ks, :],
                                    erg[:, 0:ks, 128:136], ALU.add)
                                nc.vector.tensor_tensor(
                                    sc_s[:, 0:ks, :], sc_s[:, 0:ks, :],
                                    erd[:, 0:ks, :], ALU.add)
                                nc.scalar.activation(sc_s[:, 0:ks, :],
                                                     sc_s[:, 0:ks, :],
                                                     ACTF.Prelu,
                                                     alpha=cfg.SLOPE)
                                nc.scalar.activation(ex_sb[:, ca:cz, :],
                                                     sc_s[:, 0:ks, :],
                                                     ACTF.Exp)
                                # den rides the agg matmul (dead eh cols);
                                # bias folds the (1-alpha) into 1/den
                                nc.scalar.activation(gb[:, s0:s1, D:D + 8],
                                                     sc_s[:, 0:ks, :],
                                                     ACTF.Exp, bias=lg_t[:])

                        # per-edge attention scaling: ACT expands ex to
                        # 256 cols, DVE multiplies at unit stride
                        kh = (kb + 3) // 4
                        for s0 in range(0, kb, kh):
                            s1 = min(s0 + kh, kb)
                            ks = s1 - s0
                            exx = gp.tile([128, kh, D], BF16, name="exx",
                                          tag="exx")
                            nc.scalar.activation(
                                exx[:, 0:ks, :].rearrange(
                                    "p k (h d) -> p k h d", h=H),
                                ex_sb[:, c0 + s0:c0 + s1, :].unsqueeze(3)
                                .broadcast_to([128, ks, H, AD]),
                                ACTF.Copy)
                            nc.vector.tensor_tensor(
                                gb[:, s0:s1, 0:D], gb[:, s0:s1, 0:D],
                                exx[:, 0:ks, :], ALU.mult)

                        # segment sum via indicator matmuls
                        W_rhs = D + 8 if t == 0 else D
                        ps = pagg.tile([128, 2, 512], F32, name="agg_ps",
                                       tag="pagg")
                        for j, b in enumerate(bs):
                            chs = pr['blk_chunks'][b]
                            for ii, ci in enumerate(chs):
                                cl = ci - c0
                                nc.tensor.matmul(
                                    ps[:, j, 0:W_rhs],
                                    ind_t[:, cl, :],
                                    gb[:, cl, 0:W_rhs],
                                    start=(ii == 0),
                                    stop=(ii == len(chs) - 1))

                        b0 = bs[0]
                        if t == 0:
                            nc.vector.reciprocal(rden_sb[:, b0:b0 + nb, :],
                                                 ps[:, 0:nb, D:D + 8])

                        f0s_ld = gp.tile([128, 2, D], BF16, name="f0s_ld",
                                         tag="f0s_ld")
                        nc.sync.dma_start(
                            f0s_ld[:, 0:nb, :],
                            feat0s_d.ap()[b0 * 128:(b0 + nb) * 128, :]
                            .rearrange("(j p) c -> p j c", p=128))
                        rows_t = wp.tile([128, 2, D], BF16, name="rows_t",
                                         tag="rows")
                        rd4 = (rden_sb[:, b0:b0 + nb, :].unsqueeze(3)
                               .broadcast_to([128, nb, H, AD]))
                        nc.vector.tensor_tensor(
                            rows_t[:, 0:nb, :].rearrange(
                                "p j (h d) -> p j h d", h=H),
                            ps[:, 0:nb, 0:D].rearrange(
                                "p j (h d) -> p j h d", h=H),
                            rd4, ALU.mult)
                        nc.vector.tensor_tensor(rows_t[:, 0:nb, :],
                                                rows_t[:, 0:nb, :],
                                                f0s_ld[:, 0:nb, :],
                                                ALU.add)

                        if not last:
                            r0 = b0 * 128
                            nc.sync.dma_start(
                                slabs[t + 1].ap()[r0:r0 + nb * 128, :]
                                .rearrange("(j p) c -> p j c", p=128),
                                rows_t[:, 0:nb, :])
                        else:
                            # ---- fused tail: W_out + residual + LN + FFN
                            frt = []
                            for j in range(nb):
                                frt.append(transpose_2(rows_t[:, j, :],
                                                       "frt"))
                            wo_ps = pagg.tile([128, 2, 512], F32,
                                              name="wo_ps", tag="pagg")
                            for j in range(nb):
                                for k in range(D // 128):
                                    nc.tensor.matmul(
                                        wo_ps[:, j, 0:D], frt[j][:, k, :],
                                        woc[:, k, :], start=(k == 0),
                                        stop=(k == D // 128 - 1))
                            r0 = b0 * 128
                            xe2 = wp.tile([128, 2, D], F32, name="xe2",
                                          tag="xe2")
                            nc.sync.dma_start(
                                xe2[:, 0:nb, :],
                                ent_own.ap()[r0:r0 + nb * 128, :]
                                .rearrange("(j p) c -> p j c", p=128))
                            rstp = wp.tile([128, 2, D], F32, name="rstp",
                                           tag="rstp")
                            nc.vector.tensor_tensor(
                                rstp[:, 0:nb, :], wo_ps[:, 0:nb, 0:D],
                                xe2[:, 0:nb, :], ALU.add)
                            xnb = wp.tile([128, 2, D], BF16, name="xnb",
                                          tag="xnb")
                            for j in range(nb):
                                ln(rstp[:, j, :], gfc[:], bfc[:],
                                   xnb[:, j, :])
                            ot = wp.tile([128, 2, D], F32, name="ot",
                                         tag="ot")
                            for j in range(nb):
                                xnt = transpose_2(xnb[:, j, :], "xnt")
                                x2t = wp.tile([128, 4 * D // 128, 128], BF16,
                                              name="x2t", tag="x2t")
                                for o in range(4 * D // 128):
                                    ps1 = ptps.tile([128, 128], F32,
                                                    name="ps1", tag="tps")
                                    for k in range(D // 128):
                                        nc.tensor.matmul(
                                            ps1[:],
                                            w1c[:, k, o * 128:(o + 1) * 128],
                                            xnt[:, k, :], start=(k == 0),
                                            stop=(k == D // 128 - 1))
                                    nc.scalar.activation(
                                        x2t[:, o, :], ps1[:], ACTF.Relu,
                                        bias=b1c[:, o:o + 1])
                                ff_ps = pmid.tile([128, D], F32,
                                                  name="ff_ps", tag="pmid")
                                for o in range(4 * D // 128):
                                    nc.tensor.matmul(
                                        ff_ps[:], x2t[:, o, :], w2c[:, o, :],
                                        start=(o == 0),
                                        stop=(o == 4 * D // 128 - 1))
                                nc.vector.tensor_tensor(
                                    ot[:, j, :], ff_ps[:], rstp[:, j, :],
                                    ALU.add)
                                nc.vector.tensor_tensor(
                                    ot[:, j, :], ot[:, j, :], b2c[:],
                                    ALU.add)
                            nc.sync.dma_start(
                                out_rows.ap()[r0:r0 + nb * 128, :]
                                .rearrange("(j p) c -> p j c", p=128),
                                ot[:, 0:nb, :])

                    if not last:
                        base = 0
                        for (p0, p1) in cfg.PIECES:
                            w = p1 - p0
                            nc.gpsimd.collective_compute(
                                "AllGather", ALU.bypass, replica_groups=RG,
                                ins=[slabs[t + 1].ap()[p0:p1, :].opt()],
                                outs=[tbls[t + 1].ap()
                                      [base:base + P * w, :].opt()])
                            base += P * w

    gi = 0
    for bb in nc.main_func.blocks:
        for inst in bb.instructions:
            if isinstance(inst, mybir.InstDMAGatherAnt):
                inst.queue_num = gi % 4
                gi += 1
    nc.finalize()
    return nc


# --------------------------------------------------------------------------
# host orchestration
# --------------------------------------------------------------------------

def make_in_maps(cfg, pl, inputs):
    P, NB, D, H = cfg.P, cfg.NB, cfg.D, cfg.H

    ent = np.asarray(inputs['ent_embed'], np.float32)
    ent_pad = np.zeros((cfg.NPAD, D), np.float32)
    ent_pad[:cfg.N] = ent
    rel = np.asarray(inputs['rel_embed'], np.float32)
    rel_pad = np.zeros((cfg.RPAD, D), np.float32)
    rel_pad[:cfg.R] = rel

    def repl(v):
        return np.tile(np.asarray(v, np.float32)[None, :], (128, 1)).copy()

    def attn_sel(a):
        a = np.asarray(a, np.float32)          # [H, AD]
        m = np.zeros((D, H), np.float32)
        c = np.arange(D)
        m[c, c // cfg.AD] = a[c // cfg.AD, c % cfg.AD]
        return m.astype(BF)

    b1 = np.asarray(inputs['b1'], np.float32).reshape(8, 128).T.copy()

    common = dict(
        w_head=np.asarray(inputs['W_head'], np.float32).astype(BF),
        w_tail=np.asarray(inputs['W_tail'], np.float32).astype(BF),
        w_ent=np.asarray(inputs['W_ent'], np.float32).astype(BF),
        w_rel=np.asarray(inputs['W_rel'], np.float32).astype(BF),
        a_h=attn_sel(inputs['attn_h']), a_t=attn_sel(inputs['attn_t']),
        a_r=attn_sel(inputs['attn_r']),
        w_out=np.asarray(inputs['W_out'], np.float32).astype(BF),
        w1=np.asarray(inputs['w1'], np.float32).astype(BF),
        w2=np.asarray(inputs['w2'], np.float32).astype(BF),
        g_e=repl(inputs['gamma_e']), be_e=repl(inputs['beta_e']),
        g_r=repl(inputs['gamma_r']), be_r=repl(inputs['beta_r']),
        g_ff=repl(inputs['gamma_ff']), be_ff=repl(inputs['beta_ff']),
        b1t=np.ascontiguousarray(b1), b2r=repl(inputs['b2']),
        rel_pad=rel_pad,
        ident_in=np.eye(128, dtype=np.float32).astype(BF),
    )

    in_maps = []
    for c in range(P):
        core = pl.cores[c]
        m = dict(common)
        m['ent_own'] = np.ascontiguousarray(ent_pad[c * NB:(c + 1) * NB])
        m['idx_feat'] = core.feat_idx
        m['idx_er'] = core.er_idx
        m['ind_in'] = core.ind
        m['indT_in'] = core.ind_T
        m['m0_in'] = core.m0
        m['m1_in'] = core.m1
        in_maps.append(m)
    return in_maps


LAST_RESULT = None


def kernel(**inputs) -> np.ndarray:
    global LAST_RESULT
    from concourse.bass_utils import run_bass_kernel_spmd
    cfg = Cfg()
    src = np.asarray(inputs['src']); dst = np.asarray(inputs['dst'])
    eids = np.asarray(inputs['e_ids'])
    pl = plan_edges(cfg, src, dst, eids)
    nc = build_nc(cfg, pl)
    in_maps = make_in_maps(cfg, pl, inputs)
    res = run_bass_kernel_spmd(nc, in_maps, core_ids=list(range(cfg.P)))
    LAST_RESULT = res
    out = np.concatenate([r['out_rows'] for r in res.results], axis=0)
    return out[:cfg.N].astype(np.float32)



# revision 20
# speedup vs baseline: 1.1633x; 1.1633x over previous
"""Trainium2 Bass kernel for a MAGNA-KG message-passing layer.

Problem: N=50000 nodes, E=800000 edges, R=500 relations, D=256 dims,
H=8 heads, 3 PPR hops.  SPMD across 8 NeuronCores.

Sharding (edge parallelism):
  * nodes range-sharded: core c owns rows [c*NB, (c+1)*NB), NB=6272
  * edges sharded by owner of dst; within a core edges are grouped per
    PAIR of 128-node dst blocks as [b0-even b1-even | b0-odd b1-odd]
    (even/odd by table-row parity -> int16 gather indices), padded to
    multiples of 128; per-(block,parity) chunk counts maxed over cores
    at build time so the SPMD instruction stream is identical.
  * per hop, per pair: one even + one odd dma_gather of src rows from a
    replicated bf16 table, host-built fp8 indicator one-hots, per-edge
    attention scaling as a single broadcast multiply (head-major
    columns -> stride-1 runs of 32), segment-sum via indicator matmuls
    in PSUM, blend with resident alpha*feat0, table republished with a
    3-piece AllGather issued inside the pair loop (2 pairs of slack) so
    the collective overlaps the remaining pairs.
  * hop0 additionally gathers eh (packed in the 768B table rows); et
    comes from an indicator-transpose matmul; er ([R, H], derived from
    the small relation weights) is baked per-slot into a resident
    constant at plan time (pad slots get -1e9 so their exp is 0); the
    softmax denominator rides the aggregation matmul for free (ex is
    written over the dead eh columns so rhs cols 256:264 sum to den).
  * the feed-forward tail is fused into hop2's pair loop; activation
    functions are phase-batched so the ACT table set never thrashes.

kernel(**inputs) takes FULL inputs and returns the FULL [N, 256] output.
"""

import numpy as np
import ml_dtypes

import concourse.bacc as bacc
import concourse.bass as bass
import concourse.mybir as mybir
from concourse import tile

F32 = mybir.dt.float32
BF16 = mybir.dt.bfloat16
FP8 = mybir.dt.float8e4
I16 = mybir.dt.int16
ALU = mybir.AluOpType
ACTF = mybir.ActivationFunctionType

BF = ml_dtypes.bfloat16

NEG_BIG = -1.0e9


class Cfg:
    def __init__(self, N=50000, E=800000, R=500, P=8, HOPS=3,
                 ALPHA=0.15, SLOPE=0.2, EPS=1e-5):
        self.N, self.E, self.R, self.P = N, E, R, P
        self.D, self.H, self.AD = 256, 8, 32
        self.HOPS, self.ALPHA, self.SLOPE, self.EPS = HOPS, ALPHA, SLOPE, EPS
        self.B = -(-N // (P * 128))          # 49 dst blocks of 128 per core
        self.NB = self.B * 128               # 6272 nodes per core (padded)
        self.NPAD = P * self.NB              # 50176
        # AllGather pieces (pair-aligned row ranges of the local slab);
        # last piece kept small so the post-loop collective tail is short
        self.PIECES = ((0, 2816), (2816, 5632), (5632, self.B * 128))
        self.SC0 = 384                       # hop0 row: [feat0(256)|eh(8)|pad]


def _cdiv(a, b):
    return -(-a // b)


# --------------------------------------------------------------------------
# host-side planning
# --------------------------------------------------------------------------

class Plan:
    pass


def row_of(cfg, node):
    """Global row in the piece-split, rank-major AllGather table layout."""
    r = node // cfg.NB
    l = node % cfg.NB
    out = np.zeros_like(node)
    base = 0
    for (p0, p1) in cfg.PIECES:
        w = p1 - p0
        m = (l >= p0) & (l < p1)
        out = np.where(m, base + r * w + (l - p0), out)
        base += cfg.P * w
    return out


def plan_edges(cfg, src, dst, eids):
    P, B, NB = cfg.P, cfg.B, cfg.NB
    src = np.asarray(src).astype(np.int64)
    dst = np.asarray(dst).astype(np.int64)
    eids = np.asarray(eids).astype(np.int64)

    core_of = dst // NB
    per_core = []
    cnts = np.zeros((P, B, 2), np.int64)
    for c in range(P):
        m = core_of == c
        s, d, r = src[m], dst[m], eids[m]
        blk = (d - c * NB) // 128
        row = row_of(cfg, s)
        par = row & 1
        order = np.lexsort((par, blk))
        s, d, r, row, blk, par = (s[order], d[order], r[order], row[order],
                                  blk[order], par[order])
        grp = blk * 2 + par
        cnt = np.bincount(grp, minlength=B * 2).reshape(B, 2)
        cnts[c] = cnt
        per_core.append((s, d, r, row, grp, cnt))

    K = np.maximum(_cdiv(cnts, 128).max(axis=0), 1)      # [B, 2] chunks

    # pairs of blocks; slot layout per pair: [evens of blocks | odds]
    pairs = [tuple(range(b, min(b + 2, B))) for b in range(0, B, 2)]
    pair_info = []
    gstart = np.zeros((B, 2), np.int64)   # slot start of (block, parity)
    ch = 0
    for bs in pairs:
        c0 = ch
        ke = int(sum(K[b, 0] for b in bs))
        ko = int(sum(K[b, 1] for b in bs))
        off = c0
        blk_chunks = {b: [] for b in bs}
        for b in bs:                       # even groups
            gstart[b, 0] = off * 128
            blk_chunks[b] += list(range(off, off + int(K[b, 0])))
            off += int(K[b, 0])
        for b in bs:                       # odd groups
            gstart[b, 1] = off * 128
            blk_chunks[b] += list(range(off, off + int(K[b, 1])))
            off += int(K[b, 1])
        ch = off
        pair_info.append(dict(bs=bs, c0=c0, ke=ke, ko=ko, kb=ke + ko,
                              blk_chunks=blk_chunks))
    TOTCH = ch
    TOT = TOTCH * 128

    pl = Plan()
    pl.K, pl.TOTCH, pl.TOT, pl.pairs = K, TOTCH, TOT, pair_info
    pl.cores = []
    for c in range(P):
        s, d, r, row, grp, cnt = per_core[c]
        starts = np.concatenate([[0], np.cumsum(cnt.reshape(-1))])[:-1]
        rank = np.arange(len(s)) - np.repeat(starts, cnt.reshape(-1))
        q = gstart.reshape(-1)[grp] + rank               # slot per edge

        feat_idx = np.zeros(TOT, np.int16)
        feat_idx[q] = (row >> 1).astype(np.int16)
        eid_slot = np.full(TOT, -1, np.int64)
        eid_slot[q] = r

        FP8NP = ml_dtypes.float8_e4m3
        ind = np.zeros((128, TOT), FP8NP)
        ind_T = np.zeros((128, TOT), FP8NP)
        lane = q % 128
        qch = q // 128
        dr = (d - c * NB) % 128
        ind[lane, qch * 128 + dr] = FP8NP(1.0)
        ind_T[dr, qch * 128 + lane] = FP8NP(1.0)

        def wrap(a):
            w = a.reshape(-1, 16).T                      # [16, TOT/16]
            return np.tile(w, (8, 1)).copy()             # [128, TOT/16]

        core = Plan()
        core.feat_idx = wrap(feat_idx)
        core.eid_slot = eid_slot
        core.ind = ind
        core.ind_T = ind_T
        pl.cores.append(core)
    return pl


# --------------------------------------------------------------------------
# bass program
# --------------------------------------------------------------------------

def build_nc(cfg, pl):
    P, B, NB, NPAD = cfg.P, cfg.B, cfg.NB, cfg.NPAD
    D, H, AD = cfg.D, cfg.H, cfg.AD
    SC0 = cfg.SC0
    TOTCH, TOT = pl.TOTCH, pl.TOT
    TOT16 = TOT // 16
    RG = [list(range(P))]

    nc = bacc.Bacc(None, target_bir_lowering=False, debug=False,
                   num_swdge_queues=4)
    shared = "Shared"

    def inp(name, shape, dtype):
        return nc.dram_tensor(name, shape, dtype, kind="ExternalInput")

    # ---- inputs -----------------------------------------------------------
    ent_own = inp("ent_own", [NB, D], F32)
    idx_feat = inp("idx_feat", [128, TOT16], I16)
    ind_in = inp("ind_in", [128, TOT], FP8)
    indT_in = inp("indT_in", [128, TOT], FP8)
    er_in = inp("er_in", [128, TOTCH * 8], F32)
    w_head = inp("w_head", [D, D], BF16)
    w_tail = inp("w_tail", [D, D], BF16)
    w_ent = inp("w_ent", [D, D], BF16)
    a_h = inp("a_h", [D, H], BF16)
    a_t = inp("a_t", [D, H], BF16)
    w_out = inp("w_out", [D, D], BF16)
    w1 = inp("w1", [D, 4 * D], BF16)
    w2 = inp("w2", [4 * D, D], BF16)
    g_e = inp("g_e", [128, D], F32)
    be_e = inp("be_e", [128, D], F32)
    g_ff = inp("g_ff", [128, D], F32)
    be_ff = inp("be_ff", [128, D], F32)
    b1t = inp("b1t", [128, 8], F32)
    b2r = inp("b2r", [128, D], F32)
    ident_in = inp("ident_in", [128, 128], BF16)

    out_rows = nc.dram_tensor("out_rows", [NB, D], F32, kind="ExternalOutput")

    # ---- internal DRAM ----------------------------------------------------
    feat0s_d = nc.dram_tensor("feat0s_d", [NB, D], BF16)
    slab0 = nc.dram_tensor("slab0", [NB, SC0], BF16)
    slab1 = nc.dram_tensor("slab1", [NB, D], BF16)
    slab2 = nc.dram_tensor("slab2", [NB, D], BF16)
    tbl0 = nc.dram_tensor("tbl0", [NPAD, SC0], BF16, addr_space=shared)
    tbl1 = nc.dram_tensor("tbl1", [NPAD, D], BF16, addr_space=shared)
    tbl2 = nc.dram_tensor("tbl2", [NPAD, D], BF16, addr_space=shared)
    slabs = [slab0, slab1, slab2]
    tbls = [tbl0, tbl1, tbl2]

    with tile.TileContext(nc, num_cores=P) as tc:
        with (
            tc.tile_pool(name="consts", bufs=1) as cp,
            tc.tile_pool(name="work", bufs=2) as wp,
            tc.tile_pool(name="gath", bufs=2) as gp,
            tc.tile_pool(name="gath3", bufs=3) as gp3,
            tc.tile_pool(name="pagg", bufs=2, space="PSUM") as pagg,
            tc.tile_pool(name="pmid", bufs=2, space="PSUM") as pmid,
            tc.tile_pool(name="ptps", bufs=2, space="PSUM") as ptps,
        ):
            from concourse import library_config
            nc.gpsimd.load_library(library_config.mlp)

            # ---- resident constants --------------------------------------
            def load_const(name, dram, shape, dtype):
                t = cp.tile(shape, dtype, name=name)
                nc.sync.dma_start(t[:], dram[:, :])
                return t

            ident = load_const("identc", ident_in, [128, 128], BF16)
            erc = cp.tile([128, TOTCH, 8], F32, name="erc")
            nc.sync.dma_start(erc[:].rearrange("p k h -> p (k h)"),
                              er_in[:, :])

            def load_w(name, dram, cols):
                t = cp.tile([128, D // 128, cols], BF16, name=name)
                nc.sync.dma_start(
                    t[:], dram.ap().rearrange("(kt p) c -> p kt c", p=128))
                return t

            whc = load_w("whc", w_head, D)
            wtc = load_w("wtc", w_tail, D)
            wec = load_w("wec", w_ent, D)
            ahc = load_w("ahc", a_h, H)
            atc = load_w("atc", a_t, H)
            woc = load_w("woc", w_out, D)
            w1c = load_w("w1c", w1, 4 * D)
            w2c = cp.tile([128, 4 * D // 128, D], BF16, name="w2c")
            nc.sync.dma_start(
                w2c[:], w2.ap().rearrange("(kt p) c -> p kt c", p=128))
            gec = load_const("gec", g_e, [128, D], F32)
            bec = load_const("bec", be_e, [128, D], F32)
            gfc = load_const("gfc", g_ff, [128, D], F32)
            bfc = load_const("bfc", be_ff, [128, D], F32)
            b1c = load_const("b1c", b1t, [128, 8], F32)
            b2c = load_const("b2c", b2r, [128, D], F32)

            ex_sb = cp.tile([128, TOTCH, 8], BF16, name="ex_sb")
            rden_sb = cp.tile([128, B, 8], F32, name="rden_sb")
            et_own = cp.tile([128, B, 8], BF16, name="et_own")
            eps_t = cp.tile([128, 1], F32, name="eps_t")
            nc.vector.memset(eps_t[:], cfg.EPS)
            lg_t = cp.tile([128, 1], F32, name="lg_t")
            nc.vector.memset(lg_t[:], float(-np.log(1.0 - cfg.ALPHA)))

            # ------------------------------------------------------------------
            # helpers
            # ------------------------------------------------------------------
            def ln(x_f32, gamma, beta, out_t):
                st = wp.tile([128, 6], F32, name="ln_st", tag="ln_st")
                ag = wp.tile([128, 2], F32, name="ln_ag", tag="ln_ag")
                sd = wp.tile([128, 1], F32, name="ln_sd", tag="ln_sd")
                rv = wp.tile([128, 1], F32, name="ln_rv", tag="ln_rv")
                xc = wp.tile([128, D], F32, name="ln_xc", tag="ln_xc")
                nc.vector.bn_stats(st[:], x_f32)
                nc.vector.bn_aggr(ag[:], st[:])
                nc.scalar.activation(sd[:], ag[:, 1:2], ACTF.Sqrt,
                                     bias=eps_t[:])
                nc.vector.reciprocal(rv[:], sd[:])
                nc.vector.tensor_scalar(xc[:], x_f32, ag[:, 0:1], rv[:],
                                        ALU.subtract, ALU.mult)
                nc.vector.scalar_tensor_tensor(
                    xc[:], xc[:], 1.0, gamma, ALU.mult, ALU.mult)
                nc.vector.tensor_tensor(out_t, xc[:], beta, ALU.add)

            def transpose_2(src_bf16, name):
                """[128, D] bf16 -> [128, 2, 128] transposed k-tiles."""
                t = wp.tile([128, D // 128, 128], BF16, name=name, tag="tps_o")
                for k in range(D // 128):
                    ps = ptps.tile([128, 128], BF16, name="tps_ps", tag="tps")
                    nc.tensor.transpose(
                        ps[:], src_bf16[:, k * 128:(k + 1) * 128], ident[:])
                    nc.vector.tensor_copy(t[:, k, :], ps[:])
                return t

            def gather(out_t, tbl_view, idx_dram, q0, n, elem, estep, name):
                it = gp.tile([128, n // 16], I16, name=name, tag=name)
                nc.sync.dma_start(it[:], idx_dram[:, q0 // 16:(q0 + n) // 16])
                nc.gpsimd.dma_gather(out_t, tbl_view, it[:], n, n, elem,
                                     elem_step=estep, single_packet=False)

            # ------------------------------------------------------------------
            # P1a: entity LayerNorms (ACT set: sqrt)
            # ------------------------------------------------------------------
            with nc.named_scope("p1a"):
                he_all = gp3.tile([128, B, D], BF16, name="he_all",
                                  tag="gb")
                for i in range(B):
                    xe = wp.tile([128, D], F32, name="xe", tag="x_in")
                    nc.sync.dma_start(xe[:], ent_own[i * 128:(i + 1) * 128, :])
                    ln(xe[:], gec[:], bec[:], he_all[:, i, :])

            # ------------------------------------------------------------------
            # P1b: head — projections, eh/et, feat0  (ACT set: tanh)
            # ------------------------------------------------------------------
            with nc.named_scope("p1b"):
                for i in range(B):
                    het = transpose_2(he_all[:, i, :], "het")
                    f0r = wp.tile([128, SC0], BF16, name="f0r", tag="f0r")
                    nc.vector.memset(f0r[:, D:], 0.0)
                    for (wc, ac, sl) in ((whc, ahc, 0), (wtc, atc, 1)):
                        tht = wp.tile([128, D // 128, 128], BF16, name="thx",
                                      tag="tht")
                        for o in range(D // 128):
                            ps = ptps.tile([128, 128], F32, name="pp",
                                           tag="tps")
                            for k in range(D // 128):
                                nc.tensor.matmul(
                                    ps[:], wc[:, k, o * 128:(o + 1) * 128],
                                    het[:, k, :], start=(k == 0),
                                    stop=(k == D // 128 - 1))
                            nc.scalar.activation(tht[:, o, :], ps[:],
                                                 ACTF.Tanh)
                        ap_ps = pmid.tile([16, 128], F32, name="ap_ps",
                                          tag="pmid")
                        for o in range(D // 128):
                            nc.tensor.matmul(ap_ps[0:8, :], ac[:, o, :],
                                             tht[:, o, :], start=(o == 0),
                                             stop=(o == D // 128 - 1))
                        aps = wp.tile([8, 128], BF16, name="aps", tag="ers")
                        nc.vector.tensor_copy(aps[:], ap_ps[0:8, :])
                        spt = ptps.tile([128, 128], BF16, name="spt",
                                        tag="tps")
                        nc.tensor.transpose(spt[:, 0:8], aps[:],
                                            ident[0:8, 0:8])
                        if sl == 0:
                            nc.vector.tensor_copy(f0r[:, D:D + 8],
                                                  spt[:, 0:8])
                        else:
                            nc.vector.tensor_copy(et_own[:, i, :],
                                                  spt[:, 0:8])

                    f0t = wp.tile([128, D // 128, 128], BF16, name="f0t",
                                  tag="tht")
                    for o in range(D // 128):
                        ps = ptps.tile([128, 128], F32, name="fp", tag="tps")
                        for k in range(D // 128):
                            nc.tensor.matmul(
                                ps[:], wec[:, k, o * 128:(o + 1) * 128],
                                het[:, k, :], start=(k == 0),
                                stop=(k == D // 128 - 1))
                        nc.vector.tensor_copy(f0t[:, o, :], ps[:])
                    for o in range(D // 128):
                        ps = ptps.tile([128, 128], BF16, name="fr", tag="tps")
                        nc.tensor.transpose(ps[:], f0t[:, o, :], ident[:])
                        nc.vector.tensor_copy(f0r[:, o * 128:(o + 1) * 128],
                                              ps[:])
                    nc.sync.dma_start(slab0[i * 128:(i + 1) * 128, :], f0r[:])
                    f0s = wp.tile([128, D], BF16, name="f0s", tag="f0s")
                    nc.vector.tensor_scalar_mul(f0s[:], f0r[:, 0:D],
                                                cfg.ALPHA)
                    nc.sync.dma_start(feat0s_d[i * 128:(i + 1) * 128, :],
                                      f0s[:])
                    # publish each piece as soon as its blocks (+2 slack)
                    # are written so the AllGather overlaps the block loop
                    for pno, (p0, p1) in enumerate(cfg.PIECES):
                        if i == min(p1 // 128 - 1 + 2, B - 1):
                            w = p1 - p0
                            base = P * sum(q1 - q0 for (q0, q1)
                                           in cfg.PIECES[:pno])
                            nc.gpsimd.collective_compute(
                                "AllGather", ALU.bypass, replica_groups=RG,
                                ins=[slab0.ap()[p0:p1, :].opt()],
                                outs=[tbl0.ap()
                                      [base:base + P * w, :].opt()])

            # ------------------------------------------------------------------
            # hops
            # ------------------------------------------------------------------
            npairs = len(pl.pairs)
            for t in range(cfg.HOPS):
                W = SC0 if t == 0 else D
                tb_v = tbls[t].ap().rearrange("(n two) c -> n (two c)", two=2)
                tb_even, tb_odd = tb_v[:, 0:W], tb_v[:, W:2 * W]
                last = t + 1 == cfg.HOPS
                with nc.named_scope(f"hop{t}"):
                    for pi, pr in enumerate(pl.pairs):
                        bs, c0, ke, ko, kb = (pr['bs'], pr['c0'], pr['ke'],
                                              pr['ko'], pr['kb'])
                        nb = len(bs)
                        q0 = c0 * 128

                        gb = gp3.tile([128, kb, W], BF16, name="gb",
                                      tag="gb")
                        gather(gb[:, 0:ke, :], tb_even, idx_feat, q0,
                               ke * 128, W, 2 * W, "ix_f0")
                        gather(gb[:, ke:kb, :], tb_odd, idx_feat,
                               q0 + ke * 128, ko * 128, W, 2 * W, "ix_f1")

                        ind_t = gp.tile([128, kb, 128], FP8, name="ind_t",
                                        tag="ind_t")
                        nc.sync.dma_start(
                            ind_t[:].rearrange("p k l -> p (k l)"),
                            ind_in[:, q0:q0 + kb * 128])

                        if t == 0:
                            kh0 = (kb + 1) // 2
                            for s0 in range(0, kb, kh0):
                                s1 = min(s0 + kh0, kb)
                                ks = s1 - s0
                                ca, cz = c0 + s0, c0 + s1
                                qa = ca * 128
                                indT_t = gp.tile([128, kh0 * 128], FP8,
                                                 name="indT_t", tag="indT_t")
                                nc.sync.dma_start(
                                    indT_t[:, 0:ks * 128],
                                    indT_in[:, qa:qa + ks * 128])
                                et_ps = pmid.tile([128, kh0, 8], F32,
                                                  name="et_ps", tag="pmid")
                                for ci in range(ks):
                                    b_of = next(
                                        b for b in bs
                                        if ca + ci in pr['blk_chunks'][b])
                                    nc.tensor.matmul(
                                        et_ps[:, ci, :],
                                        indT_t[:, ci * 128:(ci + 1) * 128],
                                        et_own[:, b_of, :],
                                        start=True, stop=True)
                                sc_s = wp.tile([128, kh0, 8], F32,
                                               name="sc_s", tag="sc_s")
                                nc.vector.tensor_tensor(
                                    sc_s[:, 0:ks, :], gb[:, s0:s1, D:D + 8],
                                    et_ps[:, 0:ks, :], ALU.add)
                                nc.vector.tensor_tensor(
                                    sc_s[:, 0:ks, :], sc_s[:, 0:ks, :],
                                    erc[:, ca:cz, :], ALU.add)
                                nc.scalar.activation(sc_s[:, 0:ks, :],
                                                     sc_s[:, 0:ks, :],
                                                     ACTF.Prelu,
                                                     alpha=cfg.SLOPE)
                                nc.scalar.activation(ex_sb[:, ca:cz, :],
                                                     sc_s[:, 0:ks, :],
                                                     ACTF.Exp)
                                # den rides the agg matmul (dead eh cols);
                                # bias folds the (1-alpha) into 1/den
                                nc.scalar.activation(gb[:, s0:s1, D:D + 8],
                                                     sc_s[:, 0:ks, :],
                                                     ACTF.Exp, bias=lg_t[:])

                        # per-edge attention scaling: ACT expands ex to
                        # 256 cols, DVE multiplies at unit stride
                        kh = (kb + 3) // 4
                        for s0 in range(0, kb, kh):
                            s1 = min(s0 + kh, kb)
                            ks = s1 - s0
                            exx = gp.tile([128, kh, D], BF16, name="exx",
                                          tag="exx")
                            nc.scalar.activation(
                                exx[:, 0:ks, :].rearrange(
                                    "p k (h d) -> p k h d", h=H),
                                ex_sb[:, c0 + s0:c0 + s1, :].unsqueeze(3)
                                .broadcast_to([128, ks, H, AD]),
                                ACTF.Copy)
                            nc.vector.tensor_tensor(
                                gb[:, s0:s1, 0:D], gb[:, s0:s1, 0:D],
                                exx[:, 0:ks, :], ALU.mult)

                        # segment sum via indicator matmuls
                        W_rhs = D + 8 if t == 0 else D
                        ps = pagg.tile([128, 2, 512], F32, name="agg_ps",
                                       tag="pagg")
                        for j, b in enumerate(bs):
                            chs = pr['blk_chunks'][b]
                            for ii, ci in enumerate(chs):
                                cl = ci - c0
                                nc.tensor.matmul(
                                    ps[:, j, 0:W_rhs],
                                    ind_t[:, cl, :],
                                    gb[:, cl, 0:W_rhs],
                                    start=(ii == 0),
                                    stop=(ii == len(chs) - 1))

                        b0 = bs[0]
                        if t == 0:
                            nc.vector.reciprocal(rden_sb[:, b0:b0 + nb, :],
                                                 ps[:, 0:nb, D:D + 8])

                        f0s_ld = gp.tile([128, 2, D], BF16, name="f0s_ld",
                                         tag="f0s_ld")
                        nc.sync.dma_start(
                            f0s_ld[:, 0:nb, :],
                            feat0s_d.ap()[b0 * 128:(b0 + nb) * 128, :]
                            .rearrange("(j p) c -> p j c", p=128))
                        rows_t = wp.tile([128, 2, D], BF16, name="rows_t",
                                         tag="rows")
                        rd4 = (rden_sb[:, b0:b0 + nb, :].unsqueeze(3)
                               .broadcast_to([128, nb, H, AD]))
                        nc.vector.tensor_tensor(
                            rows_t[:, 0:nb, :].rearrange(
                                "p j (h d) -> p j h d", h=H),
                            ps[:, 0:nb, 0:D].rearrange(
                                "p j (h d) -> p j h d", h=H),
                            rd4, ALU.mult)
                        nc.vector.tensor_tensor(rows_t[:, 0:nb, :],
                                                rows_t[:, 0:nb, :],
                                                f0s_ld[:, 0:nb, :],
                                                ALU.add)

                        if not last:
                            r0 = b0 * 128
                            nc.sync.dma_start(
                                slabs[t + 1].ap()[r0:r0 + nb * 128, :]
                                .rearrange("(j p) c -> p j c", p=128),
                                rows_t[:, 0:nb, :])
                            # publish each piece once its pairs (+2 slack)
                            # are written; overlaps the remaining pairs
                            for pno, (p0, p1) in enumerate(cfg.PIECES):
                                lastp = (p1 + 255) // 256 - 1
                                if pi == min(lastp + 2, npairs - 1):
                                    w = p1 - p0
                                    base = P * sum(
                                        q1 - q0 for (q0, q1)
                                        in cfg.PIECES[:pno])
                                    nc.gpsimd.collective_compute(
                                        "AllGather", ALU.bypass,
                                        replica_groups=RG,
                                        ins=[slabs[t + 1].ap()
                                             [p0:p1, :].opt()],
                                        outs=[tbls[t + 1].ap()
                                              [base:base + P * w, :].opt()])
                        else:
                            # ---- fused tail: W_out + residual + LN + FFN
                            frt = []
                            for j in range(nb):
                                frt.append(transpose_2(rows_t[:, j, :],
                                                       "frt"))
                            wo_ps = pagg.tile([128, 2, 512], F32,
                                              name="wo_ps", tag="pagg")
                            for j in range(nb):
                                for k in range(D // 128):
                                    nc.tensor.matmul(
                                        wo_ps[:, j, 0:D], frt[j][:, k, :],
                                        woc[:, k, :], start=(k == 0),
                                        stop=(k == D // 128 - 1))
                            r0 = b0 * 128
                            xe2 = wp.tile([128, 2, D], F32, name="xe2",
                                          tag="xe2")
                            nc.sync.dma_start(
                                xe2[:, 0:nb, :],
                                ent_own.ap()[r0:r0 + nb * 128, :]
                                .rearrange("(j p) c -> p j c", p=128))
                            rstp = wp.tile([128, 2, D], F32, name="rstp",
                                           tag="rstp")
                            nc.vector.tensor_tensor(
                                rstp[:, 0:nb, :], wo_ps[:, 0:nb, 0:D],
                                xe2[:, 0:nb, :], ALU.add)
                            xnb = wp.tile([128, 2, D], BF16, name="xnb",
                                          tag="xnb")
                            for j in range(nb):
                                ln(rstp[:, j, :], gfc[:], bfc[:],
                                   xnb[:, j, :])
                            ot = wp.tile([128, 2, D], F32, name="ot",
                                         tag="ot")
                            for j in range(nb):
                                xnt = transpose_2(xnb[:, j, :], "xnt")
                                x2t = wp.tile([128, 4 * D // 128, 128], BF16,
                                              name="x2t", tag="x2t")
                                for o in range(4 * D // 128):
                                    ps1 = ptps.tile([128, 128], F32,
                                                    name="ps1", tag="tps")
                                    for k in range(D // 128):
                                        nc.tensor.matmul(
                                            ps1[:],
                                            w1c[:, k, o * 128:(o + 1) * 128],
                                            xnt[:, k, :], start=(k == 0),
                                            stop=(k == D // 128 - 1))
                                    nc.scalar.activation(
                                        x2t[:, o, :], ps1[:], ACTF.Relu,
                                        bias=b1c[:, o:o + 1])
                                ff_ps = pmid.tile([128, D], F32,
                                                  name="ff_ps", tag="pmid")
                                for o in range(4 * D // 128):
                                    nc.tensor.matmul(
                                        ff_ps[:], x2t[:, o, :], w2c[:, o, :],
                                        start=(o == 0),
                                        stop=(o == 4 * D // 128 - 1))
                                nc.vector.tensor_tensor(
                                    ot[:, j, :], ff_ps[:], rstp[:, j, :],
                                    ALU.add)
                                nc.vector.tensor_tensor(
                                    ot[:, j, :], ot[:, j, :], b2c[:],
                                    ALU.add)
                            nc.sync.dma_start(
                                out_rows.ap()[r0:r0 + nb * 128, :]
                                .rearrange("(j p) c -> p j c", p=128),
                                ot[:, 0:nb, :])


    gi = 0
    for bb in nc.main_func.blocks:
        for inst in bb.instructions:
            if isinstance(inst, mybir.InstDMAGatherAnt):
                inst.queue_num = gi % 4
                gi += 1
    nc.finalize()
    return nc


# --------------------------------------------------------------------------
# host orchestration
# --------------------------------------------------------------------------

def make_in_maps(cfg, pl, inputs):
    P, NB, D, H = cfg.P, cfg.NB, cfg.D, cfg.H

    ent = np.asarray(inputs['ent_embed'], np.float32)
    ent_pad = np.zeros((cfg.NPAD, D), np.float32)
    ent_pad[:cfg.N] = ent

    # relation attention logits er_rel [R, H] (tiny; from the small dense
    # relation weights, so computed at plan time and baked per edge slot)
    rel = np.asarray(inputs['rel_embed'], np.float32)
    mu = rel.mean(-1, keepdims=True)
    var = ((rel - mu) ** 2).mean(-1, keepdims=True)
    h_r = (np.asarray(inputs['gamma_r'], np.float32) * (rel - mu)
           / np.sqrt(var + cfg.EPS) + np.asarray(inputs['beta_r'],
                                                 np.float32))
    rel_feat = np.tanh(h_r @ np.asarray(inputs['W_rel'], np.float32))
    er_rel = (rel_feat.reshape(cfg.R, H, cfg.AD)
              * np.asarray(inputs['attn_r'], np.float32)).sum(-1)

    def repl(v):
        return np.tile(np.asarray(v, np.float32)[None, :], (128, 1)).copy()

    def attn_sel(a):
        a = np.asarray(a, np.float32)          # [H, AD]
        m = np.zeros((D, H), np.float32)
        c = np.arange(D)
        m[c, c // cfg.AD] = a[c // cfg.AD, c % cfg.AD]
        return m.astype(BF)

    b1 = np.asarray(inputs['b1'], np.float32).reshape(8, 128).T.copy()

    common = dict(
        w_head=np.asarray(inputs['W_head'], np.float32).astype(BF),
        w_tail=np.asarray(inputs['W_tail'], np.float32).astype(BF),
        w_ent=np.asarray(inputs['W_ent'], np.float32).astype(BF),
        a_h=attn_sel(inputs['attn_h']), a_t=attn_sel(inputs['attn_t']),
        w_out=np.asarray(inputs['W_out'], np.float32).astype(BF),
        w1=np.asarray(inputs['w1'], np.float32).astype(BF),
        w2=np.asarray(inputs['w2'], np.float32).astype(BF),
        g_e=repl(inputs['gamma_e']), be_e=repl(inputs['beta_e']),
        g_ff=repl(inputs['gamma_ff']), be_ff=repl(inputs['beta_ff']),
        b1t=np.ascontiguousarray(b1), b2r=repl(inputs['b2']),
        ident_in=np.eye(128, dtype=np.float32).astype(BF),
    )

    in_maps = []
    for c in range(P):
        core = pl.cores[c]
        m = dict(common)
        m['ent_own'] = np.ascontiguousarray(ent_pad[c * NB:(c + 1) * NB])
        m['idx_feat'] = core.feat_idx
        m['ind_in'] = core.ind
        m['indT_in'] = core.ind_T
        er_slot = np.full((pl.TOT, H), NEG_BIG, np.float32)
        real = core.eid_slot >= 0
        er_slot[real] = er_rel[core.eid_slot[real]]
        m['er_in'] = np.ascontiguousarray(
            er_slot.reshape(pl.TOTCH, 128, H).transpose(1, 0, 2)
            .reshape(128, pl.TOTCH * H))
        in_maps.append(m)
    return in_maps


LAST_RESULT = None


def kernel(**inputs) -> np.ndarray:
    global LAST_RESULT
    from concourse.bass_utils import run_bass_kernel_spmd
    cfg = Cfg()
    src = np.asarray(inputs['src']); dst = np.asarray(inputs['dst'])
    eids = np.asarray(inputs['e_ids'])
    pl = plan_edges(cfg, src, dst, eids)
    nc = build_nc(cfg, pl)
    in_maps = make_in_maps(cfg, pl, inputs)
    res = run_bass_kernel_spmd(nc, in_maps, core_ids=list(range(cfg.P)))
    LAST_RESULT = res
    out = np.concatenate([r['out_rows'] for r in res.results], axis=0)
    return out[:cfg.N].astype(np.float32)



# revision 28
# speedup vs baseline: 1.1930x; 1.0256x over previous
"""Trainium2 Bass kernel for a MAGNA-KG message-passing layer.

Problem: N=50000 nodes, E=800000 edges, R=500 relations, D=256 dims,
H=8 heads, 3 PPR hops.  SPMD across 8 NeuronCores.

Sharding (edge parallelism):
  * nodes range-sharded: core c owns rows [c*NB, (c+1)*NB), NB=6272
  * edges sharded by owner of dst; within a core edges are grouped per
    PAIR of 128-node dst blocks as [b0-even b1-even | b0-odd b1-odd]
    (even/odd by table-row parity -> int16 gather indices), padded to
    multiples of 128; per-(block,parity) chunk counts maxed over cores
    at build time so the SPMD instruction stream is identical.
  * per hop, per pair: one even + one odd dma_gather of src rows from a
    replicated bf16 table, host-built fp8 indicator one-hots, per-edge
    attention scaling as a single broadcast multiply (head-major
    columns -> stride-1 runs of 32), segment-sum via indicator matmuls
    in PSUM, blend with resident alpha*feat0, table republished with a
    3-piece AllGather issued inside the pair loop (2 pairs of slack) so
    the collective overlaps the remaining pairs.
  * hop0 additionally gathers eh (packed in the 768B table rows); et
    comes from an indicator-transpose matmul; er ([R, H], derived from
    the small relation weights) is baked per-slot into a resident
    constant at plan time (pad slots get -1e9 so their exp is 0); the
    softmax denominator rides the aggregation matmul for free (ex is
    written over the dead eh columns so rhs cols 256:264 sum to den).
  * the feed-forward tail is fused into hop2's pair loop; activation
    functions are phase-batched so the ACT table set never thrashes.

kernel(**inputs) takes FULL inputs and returns the FULL [N, 256] output.
"""

import numpy as np
import ml_dtypes

import concourse.bacc as bacc
import concourse.bass as bass
import concourse.mybir as mybir
from concourse import tile

F32 = mybir.dt.float32
BF16 = mybir.dt.bfloat16
FP8 = mybir.dt.float8e4
I16 = mybir.dt.int16
ALU = mybir.AluOpType
ACTF = mybir.ActivationFunctionType

BF = ml_dtypes.bfloat16

NEG_BIG = -1.0e9


class Cfg:
    def __init__(self, N=50000, E=800000, R=500, P=8, HOPS=3,
                 ALPHA=0.15, SLOPE=0.2, EPS=1e-5):
        self.N, self.E, self.R, self.P = N, E, R, P
        self.D, self.H, self.AD = 256, 8, 32
        self.HOPS, self.ALPHA, self.SLOPE, self.EPS = HOPS, ALPHA, SLOPE, EPS
        self.B = -(-N // (P * 128))          # 49 dst blocks of 128 per core
        self.NB = self.B * 128               # 6272 nodes per core (padded)
        self.NPAD = P * self.NB              # 50176
        # AllGather pieces (pair-aligned row ranges of the local slab);
        # front-heavy so most collective bytes overlap the pair loop and
        # the post-loop tail is short
        self.PIECES = ((0, 3584), (3584, 5888), (5888, self.B * 128))
        self.SC0 = 384                       # hop0 row: [feat0(256)|eh(8)|pad]


def _cdiv(a, b):
    return -(-a // b)


# --------------------------------------------------------------------------
# host-side planning
# --------------------------------------------------------------------------

class Plan:
    pass


def row_of(cfg, node):
    """Global row in the piece-split, rank-major AllGather table layout."""
    r = node // cfg.NB
    l = node % cfg.NB
    out = np.zeros_like(node)
    base = 0
    for (p0, p1) in cfg.PIECES:
        w = p1 - p0
        m = (l >= p0) & (l < p1)
        out = np.where(m, base + r * w + (l - p0), out)
        base += cfg.P * w
    return out


def plan_edges(cfg, src, dst, eids):
    P, B, NB = cfg.P, cfg.B, cfg.NB
    src = np.asarray(src).astype(np.int64)
    dst = np.asarray(dst).astype(np.int64)
    eids = np.asarray(eids).astype(np.int64)

    core_of = dst // NB
    per_core = []
    cnts = np.zeros((P, B, 2), np.int64)
    for c in range(P):
        m = core_of == c
        s, d, r = src[m], dst[m], eids[m]
        blk = (d - c * NB) // 128
        row = row_of(cfg, s)
        par = row & 1
        order = np.lexsort((par, blk))
        s, d, r, row, blk, par = (s[order], d[order], r[order], row[order],
                                  blk[order], par[order])
        grp = blk * 2 + par
        cnt = np.bincount(grp, minlength=B * 2).reshape(B, 2)
        cnts[c] = cnt
        per_core.append((s, d, r, row, grp, cnt))

    K = np.maximum(_cdiv(cnts, 128).max(axis=0), 1)      # [B, 2] chunks

    # pairs of blocks; slot layout per pair: [evens of blocks | odds]
    pairs = [tuple(range(b, min(b + 2, B))) for b in range(0, B, 2)]
    pair_info = []
    gstart = np.zeros((B, 2), np.int64)   # slot start of (block, parity)
    ch = 0
    for bs in pairs:
        c0 = ch
        ke = int(sum(K[b, 0] for b in bs))
        ko = int(sum(K[b, 1] for b in bs))
        off = c0
        blk_chunks = {b: [] for b in bs}
        for b in bs:                       # even groups
            gstart[b, 0] = off * 128
            blk_chunks[b] += list(range(off, off + int(K[b, 0])))
            off += int(K[b, 0])
        for b in bs:                       # odd groups
            gstart[b, 1] = off * 128
            blk_chunks[b] += list(range(off, off + int(K[b, 1])))
            off += int(K[b, 1])
        ch = off
        pair_info.append(dict(bs=bs, c0=c0, ke=ke, ko=ko, kb=ke + ko,
                              blk_chunks=blk_chunks))
    TOTCH = ch
    TOT = TOTCH * 128

    pl = Plan()
    pl.K, pl.TOTCH, pl.TOT, pl.pairs = K, TOTCH, TOT, pair_info
    pl.cores = []
    for c in range(P):
        s, d, r, row, grp, cnt = per_core[c]
        starts = np.concatenate([[0], np.cumsum(cnt.reshape(-1))])[:-1]
        rank = np.arange(len(s)) - np.repeat(starts, cnt.reshape(-1))
        q = gstart.reshape(-1)[grp] + rank               # slot per edge

        feat_idx = np.zeros(TOT, np.int16)
        feat_idx[q] = (row >> 1).astype(np.int16)
        eid_slot = np.full(TOT, -1, np.int64)
        eid_slot[q] = r

        FP8NP = ml_dtypes.float8_e4m3
        ind = np.zeros((128, TOT), FP8NP)
        ind_T = np.zeros((128, TOT), FP8NP)
        lane = q % 128
        qch = q // 128
        dr = (d - c * NB) % 128
        ind[lane, qch * 128 + dr] = FP8NP(1.0)
        ind_T[dr, qch * 128 + lane] = FP8NP(1.0)

        def wrap(a):
            w = a.reshape(-1, 16).T                      # [16, TOT/16]
            return np.tile(w, (8, 1)).copy()             # [128, TOT/16]

        core = Plan()
        core.feat_idx = wrap(feat_idx)
        core.eid_slot = eid_slot
        core.ind = ind
        core.ind_T = ind_T
        pl.cores.append(core)
    return pl


# --------------------------------------------------------------------------
# bass program
# --------------------------------------------------------------------------

def build_nc(cfg, pl):
    P, B, NB, NPAD = cfg.P, cfg.B, cfg.NB, cfg.NPAD
    D, H, AD = cfg.D, cfg.H, cfg.AD
    SC0 = cfg.SC0
    TOTCH, TOT = pl.TOTCH, pl.TOT
    TOT16 = TOT // 16
    RG = [list(range(P))]

    nc = bacc.Bacc(None, target_bir_lowering=False, debug=False,
                   num_swdge_queues=4)
    shared = "Shared"

    def inp(name, shape, dtype):
        return nc.dram_tensor(name, shape, dtype, kind="ExternalInput")

    # ---- inputs -----------------------------------------------------------
    ent_own = inp("ent_own", [NB, D], F32)
    idx_feat = inp("idx_feat", [128, TOT16], I16)
    ind_in = inp("ind_in", [128, TOT], FP8)
    indT_in = inp("indT_in", [128, TOT], FP8)
    er_in = inp("er_in", [128, TOTCH * 8], BF16)
    w_head = inp("w_head", [D, D], BF16)
    w_tail = inp("w_tail", [D, D], BF16)
    w_ent = inp("w_ent", [D, D], BF16)
    a_h = inp("a_h", [D, H], BF16)
    a_t = inp("a_t", [D, H], BF16)
    w_out = inp("w_out", [D, D], BF16)
    w1 = inp("w1", [D, 4 * D], BF16)
    w2 = inp("w2", [4 * D, D], BF16)
    g_e = inp("g_e", [128, D], F32)
    be_e = inp("be_e", [128, D], F32)
    g_ff = inp("g_ff", [128, D], F32)
    be_ff = inp("be_ff", [128, D], F32)
    b1t = inp("b1t", [128, 8], F32)
    b2r = inp("b2r", [128, D], F32)
    ident_in = inp("ident_in", [128, 128], BF16)

    out_rows = nc.dram_tensor("out_rows", [NB, D], F32, kind="ExternalOutput")

    # ---- internal DRAM ----------------------------------------------------
    feat0s_d = nc.dram_tensor("feat0s_d", [NB, D], BF16)
    slab0 = nc.dram_tensor("slab0", [NB, SC0], BF16)
    slab1 = nc.dram_tensor("slab1", [NB, D], BF16)
    slab2 = nc.dram_tensor("slab2", [NB, D], BF16)
    tbl0 = nc.dram_tensor("tbl0", [NPAD, SC0], BF16, addr_space=shared)
    tbl1 = nc.dram_tensor("tbl1", [NPAD, D], BF16, addr_space=shared)
    tbl2 = nc.dram_tensor("tbl2", [NPAD, D], BF16, addr_space=shared)
    slabs = [slab0, slab1, slab2]
    tbls = [tbl0, tbl1, tbl2]

    with tile.TileContext(nc, num_cores=P) as tc:
        with (
            tc.tile_pool(name="consts", bufs=1) as cp,
            tc.tile_pool(name="work", bufs=2) as wp,
            tc.tile_pool(name="gath", bufs=2) as gp,
            tc.tile_pool(name="gath3", bufs=3) as gp3,
            tc.tile_pool(name="pagg", bufs=2, space="PSUM") as pagg,
            tc.tile_pool(name="pmid", bufs=2, space="PSUM") as pmid,
            tc.tile_pool(name="ptps", bufs=2, space="PSUM") as ptps,
        ):
            from concourse import library_config
            nc.gpsimd.load_library(library_config.mlp)

            # ---- resident constants --------------------------------------
            def load_const(name, dram, shape, dtype):
                t = cp.tile(shape, dtype, name=name)
                nc.sync.dma_start(t[:], dram[:, :])
                return t

            ident = load_const("identc", ident_in, [128, 128], BF16)
            erc = cp.tile([128, TOTCH, 8], BF16, name="erc")
            nc.sync.dma_start(erc[:].rearrange("p k h -> p (k h)"),
                              er_in[:, :])

            def load_w(name, dram, cols):
                t = cp.tile([128, D // 128, cols], BF16, name=name)
                nc.sync.dma_start(
                    t[:], dram.ap().rearrange("(kt p) c -> p kt c", p=128))
                return t

            whc = load_w("whc", w_head, D)
            wtc = load_w("wtc", w_tail, D)
            wec = load_w("wec", w_ent, D)
            ahc = load_w("ahc", a_h, H)
            atc = load_w("atc", a_t, H)
            woc = load_w("woc", w_out, D)
            w1c = load_w("w1c", w1, 4 * D)
            w2c = cp.tile([128, 4 * D // 128, D], BF16, name="w2c")
            nc.sync.dma_start(
                w2c[:], w2.ap().rearrange("(kt p) c -> p kt c", p=128))
            gec = load_const("gec", g_e, [128, D], F32)
            bec = load_const("bec", be_e, [128, D], F32)
            gfc = load_const("gfc", g_ff, [128, D], F32)
            bfc = load_const("bfc", be_ff, [128, D], F32)
            b1c = load_const("b1c", b1t, [128, 8], F32)
            b2c = load_const("b2c", b2r, [128, D], F32)

            ex_sb = cp.tile([128, TOTCH, 8], BF16, name="ex_sb")
            rden_sb = cp.tile([128, B, 8], F32, name="rden_sb")
            et_own = cp.tile([128, B, 8], BF16, name="et_own")
            eps_t = cp.tile([128, 1], F32, name="eps_t")
            nc.vector.memset(eps_t[:], cfg.EPS)
            lg_t = cp.tile([128, 1], F32, name="lg_t")
            nc.vector.memset(lg_t[:], float(-np.log(1.0 - cfg.ALPHA)))

            # ------------------------------------------------------------------
            # helpers
            # ------------------------------------------------------------------
            def ln(x_f32, gamma, beta, out_t):
                st = wp.tile([128, 6], F32, name="ln_st", tag="ln_st")
                ag = wp.tile([128, 2], F32, name="ln_ag", tag="ln_ag")
                sd = wp.tile([128, 1], F32, name="ln_sd", tag="ln_sd")
                rv = wp.tile([128, 1], F32, name="ln_rv", tag="ln_rv")
                xc = wp.tile([128, D], F32, name="ln_xc", tag="ln_xc")
                nc.vector.bn_stats(st[:], x_f32)
                nc.vector.bn_aggr(ag[:], st[:])
                nc.scalar.activation(sd[:], ag[:, 1:2], ACTF.Sqrt,
                                     bias=eps_t[:])
                nc.vector.reciprocal(rv[:], sd[:])
                nc.vector.tensor_scalar(xc[:], x_f32, ag[:, 0:1], rv[:],
                                        ALU.subtract, ALU.mult)
                nc.vector.scalar_tensor_tensor(
                    xc[:], xc[:], 1.0, gamma, ALU.mult, ALU.mult)
                nc.vector.tensor_tensor(out_t, xc[:], beta, ALU.add)

            def transpose_2(src_bf16, name):
                """[128, D] bf16 -> [128, 2, 128] transposed k-tiles."""
                t = wp.tile([128, D // 128, 128], BF16, name=name, tag="tps_o")
                for k in range(D // 128):
                    ps = ptps.tile([128, 128], BF16, name="tps_ps", tag="tps")
                    nc.tensor.transpose(
                        ps[:], src_bf16[:, k * 128:(k + 1) * 128], ident[:])
                    nc.vector.tensor_copy(t[:, k, :], ps[:])
                return t

            def gather(out_t, tbl_view, idx_dram, q0, n, elem, estep, name):
                it = gp.tile([128, n // 16], I16, name=name, tag=name)
                nc.sync.dma_start(it[:], idx_dram[:, q0 // 16:(q0 + n) // 16])
                nc.gpsimd.dma_gather(out_t, tbl_view, it[:], n, n, elem,
                                     elem_step=estep, single_packet=False)

            # ------------------------------------------------------------------
            # P1a: entity LayerNorms (ACT set: sqrt)
            # ------------------------------------------------------------------
            with nc.named_scope("p1a"):
                he_all = gp3.tile([128, B, D], BF16, name="he_all",
                                  tag="gb")
                for i in range(B):
                    xe = wp.tile([128, D], F32, name="xe", tag="x_in")
                    nc.sync.dma_start(xe[:], ent_own[i * 128:(i + 1) * 128, :])
                    ln(xe[:], gec[:], bec[:], he_all[:, i, :])

            # ------------------------------------------------------------------
            # P1b: head — projections, eh/et, feat0  (ACT set: tanh)
            # ------------------------------------------------------------------
            with nc.named_scope("p1b"):
                for i in range(B):
                    het = transpose_2(he_all[:, i, :], "het")
                    f0r = wp.tile([128, SC0], BF16, name="f0r", tag="f0r")
                    nc.vector.memset(f0r[:, D:], 0.0)
                    for (wc, ac, sl) in ((whc, ahc, 0), (wtc, atc, 1)):
                        tht = wp.tile([128, D // 128, 128], BF16, name="thx",
                                      tag="tht")
                        for o in range(D // 128):
                            ps = ptps.tile([128, 128], F32, name="pp",
                                           tag="tps")
                            for k in range(D // 128):
                                nc.tensor.matmul(
                                    ps[:], wc[:, k, o * 128:(o + 1) * 128],
                                    het[:, k, :], start=(k == 0),
                                    stop=(k == D // 128 - 1))
                            nc.scalar.activation(tht[:, o, :], ps[:],
                                                 ACTF.Tanh)
                        ap_ps = pmid.tile([16, 128], F32, name="ap_ps",
                                          tag="pmid")
                        for o in range(D // 128):
                            nc.tensor.matmul(ap_ps[0:8, :], ac[:, o, :],
                                             tht[:, o, :], start=(o == 0),
                                             stop=(o == D // 128 - 1))
                        aps = wp.tile([8, 128], BF16, name="aps", tag="ers")
                        nc.vector.tensor_copy(aps[:], ap_ps[0:8, :])
                        spt = ptps.tile([128, 128], BF16, name="spt",
                                        tag="tps")
                        nc.tensor.transpose(spt[:, 0:8], aps[:],
                                            ident[0:8, 0:8])
                        if sl == 0:
                            nc.vector.tensor_copy(f0r[:, D:D + 8],
                                                  spt[:, 0:8])
                        else:
                            nc.vector.tensor_copy(et_own[:, i, :],
                                                  spt[:, 0:8])

                    f0t = wp.tile([128, D // 128, 128], BF16, name="f0t",
                                  tag="tht")
                    for o in range(D // 128):
                        ps = ptps.tile([128, 128], F32, name="fp", tag="tps")
                        for k in range(D // 128):
                            nc.tensor.matmul(
                                ps[:], wec[:, k, o * 128:(o + 1) * 128],
                                het[:, k, :], start=(k == 0),
                                stop=(k == D // 128 - 1))
                        nc.vector.tensor_copy(f0t[:, o, :], ps[:])
                    for o in range(D // 128):
                        ps = ptps.tile([128, 128], BF16, name="fr", tag="tps")
                        nc.tensor.transpose(ps[:], f0t[:, o, :], ident[:])
                        nc.vector.tensor_copy(f0r[:, o * 128:(o + 1) * 128],
                                              ps[:])
                    nc.sync.dma_start(slab0[i * 128:(i + 1) * 128, :], f0r[:])
                    f0s = wp.tile([128, D], BF16, name="f0s", tag="f0s")
                    nc.vector.tensor_scalar_mul(f0s[:], f0r[:, 0:D],
                                                cfg.ALPHA)
                    nc.sync.dma_start(feat0s_d[i * 128:(i + 1) * 128, :],
                                      f0s[:])
                    # publish each piece as soon as its blocks (+slack)
                    # are written so the AllGather overlaps the block loop
                    for pno, (p0, p1) in enumerate(cfg.PIECES):
                        slk = 2 if pno == 0 else 1
                        if i == min(p1 // 128 - 1 + slk, B - 1):
                            w = p1 - p0
                            base = P * sum(q1 - q0 for (q0, q1)
                                           in cfg.PIECES[:pno])
                            nc.gpsimd.collective_compute(
                                "AllGather", ALU.bypass, replica_groups=RG,
                                ins=[slab0.ap()[p0:p1, :].opt()],
                                outs=[tbl0.ap()
                                      [base:base + P * w, :].opt()])

            # ------------------------------------------------------------------
            # hops
            # ------------------------------------------------------------------
            npairs = len(pl.pairs)
            for t in range(cfg.HOPS):
                W = SC0 if t == 0 else D
                tb_v = tbls[t].ap().rearrange("(n two) c -> n (two c)", two=2)
                tb_even, tb_odd = tb_v[:, 0:W], tb_v[:, W:2 * W]
                last = t + 1 == cfg.HOPS
                with nc.named_scope(f"hop{t}"):
                    for pi, pr in enumerate(pl.pairs):
                        bs, c0, ke, ko, kb = (pr['bs'], pr['c0'], pr['ke'],
                                              pr['ko'], pr['kb'])
                        nb = len(bs)
                        q0 = c0 * 128

                        gb = gp3.tile([128, kb, W], BF16, name="gb",
                                      tag="gb")
                        gather(gb[:, 0:ke, :], tb_even, idx_feat, q0,
                               ke * 128, W, 2 * W, "ix_f0")
                        gather(gb[:, ke:kb, :], tb_odd, idx_feat,
                               q0 + ke * 128, ko * 128, W, 2 * W, "ix_f1")

                        ind_t = gp.tile([128, kb, 128], FP8, name="ind_t",
                                        tag="ind_t")
                        nc.sync.dma_start(
                            ind_t[:].rearrange("p k l -> p (k l)"),
                            ind_in[:, q0:q0 + kb * 128])

                        if t == 0:
                            kh0 = (kb + 1) // 2
                            for s0 in range(0, kb, kh0):
                                s1 = min(s0 + kh0, kb)
                                ks = s1 - s0
                                ca, cz = c0 + s0, c0 + s1
                                qa = ca * 128
                                indT_t = gp.tile([128, kh0 * 128], FP8,
                                                 name="indT_t", tag="indT_t")
                                nc.sync.dma_start(
                                    indT_t[:, 0:ks * 128],
                                    indT_in[:, qa:qa + ks * 128])
                                et_ps = pmid.tile([128, kh0, 8], F32,
                                                  name="et_ps", tag="pmid")
                                for ci in range(ks):
                                    b_of = next(
                                        b for b in bs
                                        if ca + ci in pr['blk_chunks'][b])
                                    nc.tensor.matmul(
                                        et_ps[:, ci, :],
                                        indT_t[:, ci * 128:(ci + 1) * 128],
                                        et_own[:, b_of, :],
                                        start=True, stop=True)
                                sc_s = wp.tile([128, kh0, 8], F32,
                                               name="sc_s", tag="sc_s")
                                nc.vector.tensor_tensor(
                                    sc_s[:, 0:ks, :], gb[:, s0:s1, D:D + 8],
                                    et_ps[:, 0:ks, :], ALU.add)
                                nc.vector.tensor_tensor(
                                    sc_s[:, 0:ks, :], sc_s[:, 0:ks, :],
                                    erc[:, ca:cz, :], ALU.add)
                                nc.scalar.activation(sc_s[:, 0:ks, :],
                                                     sc_s[:, 0:ks, :],
                                                     ACTF.Prelu,
                                                     alpha=cfg.SLOPE)
                                nc.scalar.activation(ex_sb[:, ca:cz, :],
                                                     sc_s[:, 0:ks, :],
                                                     ACTF.Exp)
                                # den rides the agg matmul (dead eh cols);
                                # bias folds the (1-alpha) into 1/den
                                nc.scalar.activation(gb[:, s0:s1, D:D + 8],
                                                     sc_s[:, 0:ks, :],
                                                     ACTF.Exp, bias=lg_t[:])

                        # per-edge attention scaling: ACT expands ex to
                        # 256 cols, DVE multiplies at unit stride
                        kh = (kb + 3) // 4
                        for s0 in range(0, kb, kh):
                            s1 = min(s0 + kh, kb)
                            ks = s1 - s0
                            exx = gp.tile([128, kh, D], BF16, name="exx",
                                          tag="exx")
                            nc.scalar.activation(
                                exx[:, 0:ks, :].rearrange(
                                    "p k (h d) -> p k h d", h=H),
                                ex_sb[:, c0 + s0:c0 + s1, :].unsqueeze(3)
                                .broadcast_to([128, ks, H, AD]),
                                ACTF.Copy)
                            nc.vector.tensor_tensor(
                                gb[:, s0:s1, 0:D], gb[:, s0:s1, 0:D],
                                exx[:, 0:ks, :], ALU.mult)

                        # segment sum via indicator matmuls
                        W_rhs = D + 8 if t == 0 else D
                        ps = pagg.tile([128, 2, 512], F32, name="agg_ps",
                                       tag="pagg")
                        for j, b in enumerate(bs):
                            chs = pr['blk_chunks'][b]
                            for ii, ci in enumerate(chs):
                                cl = ci - c0
                                nc.tensor.matmul(
                                    ps[:, j, 0:W_rhs],
                                    ind_t[:, cl, :],
                                    gb[:, cl, 0:W_rhs],
                                    start=(ii == 0),
                                    stop=(ii == len(chs) - 1))

                        b0 = bs[0]
                        if t == 0:
                            nc.vector.reciprocal(rden_sb[:, b0:b0 + nb, :],
                                                 ps[:, 0:nb, D:D + 8])

                        f0s_ld = gp.tile([128, 2, D], BF16, name="f0s_ld",
                                         tag="f0s_ld")
                        nc.sync.dma_start(
                            f0s_ld[:, 0:nb, :],
                            feat0s_d.ap()[b0 * 128:(b0 + nb) * 128, :]
                            .rearrange("(j p) c -> p j c", p=128))
                        rows_t = wp.tile([128, 2, D], BF16, name="rows_t",
                                         tag="rows")
                        rd4 = (rden_sb[:, b0:b0 + nb, :].unsqueeze(3)
                               .broadcast_to([128, nb, H, AD]))
                        nc.vector.tensor_tensor(
                            rows_t[:, 0:nb, :].rearrange(
                                "p j (h d) -> p j h d", h=H),
                            ps[:, 0:nb, 0:D].rearrange(
                                "p j (h d) -> p j h d", h=H),
                            rd4, ALU.mult)
                        nc.vector.tensor_tensor(rows_t[:, 0:nb, :],
                                                rows_t[:, 0:nb, :],
                                                f0s_ld[:, 0:nb, :],
                                                ALU.add)

                        if not last:
                            r0 = b0 * 128
                            nc.sync.dma_start(
                                slabs[t + 1].ap()[r0:r0 + nb * 128, :]
                                .rearrange("(j p) c -> p j c", p=128),
                                rows_t[:, 0:nb, :])
                            # publish each piece once its pairs (+slack)
                            # are written; overlaps the remaining pairs
                            for pno, (p0, p1) in enumerate(cfg.PIECES):
                                lastp = (p1 + 255) // 256 - 1
                                slk = 2 if pno == 0 else 1
                                if pi == min(lastp + slk, npairs - 1):
                                    w = p1 - p0
                                    base = P * sum(
                                        q1 - q0 for (q0, q1)
                                        in cfg.PIECES[:pno])
                                    nc.gpsimd.collective_compute(
                                        "AllGather", ALU.bypass,
                                        replica_groups=RG,
                                        ins=[slabs[t + 1].ap()
                                             [p0:p1, :].opt()],
                                        outs=[tbls[t + 1].ap()
                                              [base:base + P * w, :].opt()])
                        else:
                            # ---- fused tail: W_out + residual + LN + FFN
                            frt = []
                            for j in range(nb):
                                frt.append(transpose_2(rows_t[:, j, :],
                                                       "frt"))
                            wo_ps = pagg.tile([128, 2, 512], F32,
                                              name="wo_ps", tag="pagg")
                            for j in range(nb):
                                for k in range(D // 128):
                                    nc.tensor.matmul(
                                        wo_ps[:, j, 0:D], frt[j][:, k, :],
                                        woc[:, k, :], start=(k == 0),
                                        stop=(k == D // 128 - 1))
                            r0 = b0 * 128
                            xe2 = wp.tile([128, 2, D], F32, name="xe2",
                                          tag="xe2")
                            nc.sync.dma_start(
                                xe2[:, 0:nb, :],
                                ent_own.ap()[r0:r0 + nb * 128, :]
                                .rearrange("(j p) c -> p j c", p=128))
                            rstp = wp.tile([128, 2, D], F32, name="rstp",
                                           tag="rstp")
                            nc.vector.tensor_tensor(
                                rstp[:, 0:nb, :], wo_ps[:, 0:nb, 0:D],
                                xe2[:, 0:nb, :], ALU.add)
                            xnb = wp.tile([128, 2, D], BF16, name="xnb",
                                          tag="xnb")
                            for j in range(nb):
                                ln(rstp[:, j, :], gfc[:], bfc[:],
                                   xnb[:, j, :])
                            ot = wp.tile([128, 2, D], F32, name="ot",
                                         tag="ot")
                            # FFN batched over the pair's blocks: the
                            # transposed LN rows for both blocks sit side
                            # by side so FFN1 streams nb*128 cols at once
                            xnt_p = wp.tile([128, D // 128, 256], BF16,
                                            name="xnt_p", tag="xnt_p")
                            for j in range(nb):
                                for k in range(D // 128):
                                    tps = ptps.tile([128, 128], BF16,
                                                    name="xtp", tag="tps")
                                    nc.tensor.transpose(
                                        tps[:],
                                        xnb[:, j, k * 128:(k + 1) * 128],
                                        ident[:])
                                    nc.vector.tensor_copy(
                                        xnt_p[:, k, j * 128:(j + 1) * 128],
                                        tps[:])
                            x2b = wp.tile([128, 4 * D // 128, 256], BF16,
                                          name="x2b", tag="x2t")
                            for o in range(4 * D // 128):
                                ps1 = ptps.tile([128, 256], F32,
                                                name="ps1", tag="tps")
                                for k in range(D // 128):
                                    nc.tensor.matmul(
                                        ps1[:, 0:nb * 128],
                                        w1c[:, k, o * 128:(o + 1) * 128],
                                        xnt_p[:, k, 0:nb * 128],
                                        start=(k == 0),
                                        stop=(k == D // 128 - 1))
                                nc.scalar.activation(
                                    x2b[:, o, 0:nb * 128],
                                    ps1[:, 0:nb * 128], ACTF.Relu,
                                    bias=b1c[:, o:o + 1])
                            for j in range(nb):
                                ff_ps = pmid.tile([128, D], F32,
                                                  name="ff_ps", tag="pmid")
                                for o in range(4 * D // 128):
                                    nc.tensor.matmul(
                                        ff_ps[:],
                                        x2b[:, o, j * 128:(j + 1) * 128],
                                        w2c[:, o, :],
                                        start=(o == 0),
                                        stop=(o == 4 * D // 128 - 1))
                                nc.vector.tensor_tensor(
                                    ot[:, j, :], ff_ps[:], rstp[:, j, :],
                                    ALU.add)
                                nc.vector.tensor_tensor(
                                    ot[:, j, :], ot[:, j, :], b2c[:],
                                    ALU.add)
                            nc.sync.dma_start(
                                out_rows.ap()[r0:r0 + nb * 128, :]
                                .rearrange("(j p) c -> p j c", p=128),
                                ot[:, 0:nb, :])


    gi = 0
    for bb in nc.main_func.blocks:
        for inst in bb.instructions:
            if isinstance(inst, mybir.InstDMAGatherAnt):
                inst.queue_num = gi % 4
                gi += 1
    nc.finalize()
    return nc


# --------------------------------------------------------------------------
# host orchestration
# --------------------------------------------------------------------------

def make_in_maps(cfg, pl, inputs):
    P, NB, D, H = cfg.P, cfg.NB, cfg.D, cfg.H

    ent = np.asarray(inputs['ent_embed'], np.float32)
    ent_pad = np.zeros((cfg.NPAD, D), np.float32)
    ent_pad[:cfg.N] = ent

    # relation attention logits er_rel [R, H] (tiny; from the small dense
    # relation weights, so computed at plan time and baked per edge slot)
    rel = np.asarray(inputs['rel_embed'], np.float32)
    mu = rel.mean(-1, keepdims=True)
    var = ((rel - mu) ** 2).mean(-1, keepdims=True)
    h_r = (np.asarray(inputs['gamma_r'], np.float32) * (rel - mu)
           / np.sqrt(var + cfg.EPS) + np.asarray(inputs['beta_r'],
                                                 np.float32))
    rel_feat = np.tanh(h_r @ np.asarray(inputs['W_rel'], np.float32))
    er_rel = (rel_feat.reshape(cfg.R, H, cfg.AD)
              * np.asarray(inputs['attn_r'], np.float32)).sum(-1)

    def repl(v):
        return np.tile(np.asarray(v, np.float32)[None, :], (128, 1)).copy()

    def attn_sel(a):
        a = np.asarray(a, np.float32)          # [H, AD]
        m = np.zeros((D, H), np.float32)
        c = np.arange(D)
        m[c, c // cfg.AD] = a[c // cfg.AD, c % cfg.AD]
        return m.astype(BF)

    b1 = np.asarray(inputs['b1'], np.float32).reshape(8, 128).T.copy()

    common = dict(
        w_head=np.asarray(inputs['W_head'], np.float32).astype(BF),
        w_tail=np.asarray(inputs['W_tail'], np.float32).astype(BF),
        w_ent=np.asarray(inputs['W_ent'], np.float32).astype(BF),
        a_h=attn_sel(inputs['attn_h']), a_t=attn_sel(inputs['attn_t']),
        w_out=np.asarray(inputs['W_out'], np.float32).astype(BF),
        w1=np.asarray(inputs['w1'], np.float32).astype(BF),
        w2=np.asarray(inputs['w2'], np.float32).astype(BF),
        g_e=repl(inputs['gamma_e']), be_e=repl(inputs['beta_e']),
        g_ff=repl(inputs['gamma_ff']), be_ff=repl(inputs['beta_ff']),
        b1t=np.ascontiguousarray(b1), b2r=repl(inputs['b2']),
        ident_in=np.eye(128, dtype=np.float32).astype(BF),
    )

    in_maps = []
    for c in range(P):
        core = pl.cores[c]
        m = dict(common)
        m['ent_own'] = np.ascontiguousarray(ent_pad[c * NB:(c + 1) * NB])
        m['idx_feat'] = core.feat_idx
        m['ind_in'] = core.ind
        m['indT_in'] = core.ind_T
        er_slot = np.full((pl.TOT, H), NEG_BIG, np.float32)
        real = core.eid_slot >= 0
        er_slot[real] = er_rel[core.eid_slot[real]]
        m['er_in'] = np.ascontiguousarray(
            er_slot.reshape(pl.TOTCH, 128, H).transpose(1, 0, 2)
            .reshape(128, pl.TOTCH * H)).astype(BF)
        in_maps.append(m)
    return in_maps


LAST_RESULT = None


def kernel(**inputs) -> np.ndarray:
    global LAST_RESULT
    from concourse.bass_utils import run_bass_kernel_spmd
    cfg = Cfg()
    src = np.asarray(inputs['src']); dst = np.asarray(inputs['dst'])
    eids = np.asarray(inputs['e_ids'])
    pl = plan_edges(cfg, src, dst, eids)
    nc = build_nc(cfg, pl)
    in_maps = make_in_maps(cfg, pl, inputs)
    res = run_bass_kernel_spmd(nc, in_maps, core_ids=list(range(cfg.P)))
    LAST_RESULT = res
    out = np.concatenate([r['out_rows'] for r in res.results], axis=0)
    return out[:cfg.N].astype(np.float32)

